# revision 35
# baseline (speedup 1.0000x reference)
"""Expert-choice token-sparse MoE for Trainium2 (8 NeuronCores, expert-parallel).

Contract: kernel(**inputs) takes the FULL unsharded inputs and returns the FULL
output, matching reference.reference(): a tuple (final [B,S,D] f32, idx [E,K] i32).

Sharding strategy (expert-parallel, per the hint):
  - The router + top-k runs on host CPU (bit-exact replication of the oracle's
    jax ops — idx is an integer output that must match exactly, which requires
    bit-identical fp32 routing scores; the heavy compute is NOT here).
  - Expert e's gathered tokens + weights are shipped to core e ("all-to-all
    dispatch" done host-side as part of sharding).
  - Each core runs the dense MLP for its 8192 selected tokens on the Tensor
    engine in fp8e4 DoubleRow (2 fp8 MACs/cell/cycle):
    hT = relu(w1^T @ xgT + b1); out = (hT^T @ w2) * gate  (~137 GFLOP/core).
    The fp8 quantization error is attenuated by the gate (~1e-4) relative to
    the fp32 residual, so final relative error stays ~2e-5. A bf16 variant
    handles the (spec-wise impossible) b2 != 0 case.
  - Host combines: scatter-add each expert's compact [K, D] output + residual.
Measured: ~911 us HW exec per core (92% MFU; fp8 roofline for the 137 GFLOP
is ~874 us + ~14 us DMA head + ~12 us drain tail).
"""

import numpy as np
import ml_dtypes

E = 8
D = 1024
F = 4096
N = 32768
K = 8192
P = 128
TB = 256              # tokens per device block
NDC = D // P          # 8 d-chunks
NFT = F // P          # 32 f-tiles
NB = K // TB          # 32 token blocks
NTS = TB // P         # 2 token subtiles per block
DH = 512              # out D half (PSUM bank)
NDH = D // DH         # 2

_STATE = {}


def build_nc(has_b2=False, nb=NB):
    import concourse.bacc as bacc
    import concourse.mybir as mybir
    import concourse.tile as tile

    bf16 = mybir.dt.bfloat16
    f32 = mybir.dt.float32
    Relu = mybir.ActivationFunctionType.Relu

    k = nb * TB
    nc = bacc.Bacc("TRN2", target_bir_lowering=False, debug=False)
    xgT = nc.dram_tensor("xgT", [D, k], bf16, kind="ExternalInput")
    w1 = nc.dram_tensor("w1", [D, F], bf16, kind="ExternalInput")
    w2 = nc.dram_tensor("w2", [F, D], bf16, kind="ExternalInput")
    b1t = nc.dram_tensor("b1t", [P, NFT], f32, kind="ExternalInput")
    gt = nc.dram_tensor("gt", [P, k // P], f32, kind="ExternalInput")
    if has_b2:
        b2r = nc.dram_tensor("b2r", [1, D], bf16, kind="ExternalInput")
    out = nc.dram_tensor("out", [k, D], bf16, kind="ExternalOutput")

    with tile.TileContext(nc) as tc:
        with (
            tc.tile_pool(name="wpool", bufs=1) as wpool,
            tc.tile_pool(name="xpool", bufs=3) as xpool,
            tc.tile_pool(name="hpool", bufs=2) as hpool,
            tc.tile_pool(name="opool", bufs=3) as opool,
            tc.tile_pool(name="pp1", bufs=4, space="PSUM") as pp1,
            tc.tile_pool(name="pp2", bufs=4, space="PSUM") as pp2,
        ):
            # resident weights
            w1_sb = []
            for c in range(NDC):
                t = wpool.tile([P, F], bf16, tag=f"w1_{c}")
                nc.sync.dma_start(t[:], w1.ap()[c * P:(c + 1) * P, :])
                w1_sb.append(t)
            w2_sb = []
            for c in range(NFT):
                t = wpool.tile([P, D], bf16, tag=f"w2_{c}")
                nc.sync.dma_start(t[:], w2.ap()[c * P:(c + 1) * P, :])
                w2_sb.append(t)
            b1_sb = wpool.tile([P, NFT], f32, tag="b1")
            nc.sync.dma_start(b1_sb[:], b1t.ap())
            g_sb = wpool.tile([P, k // P], f32, tag="g")
            nc.sync.dma_start(g_sb[:], gt.ap())
            if has_b2:
                b2_sb = wpool.tile([1, D], bf16, tag="b2")
                nc.sync.dma_start(b2_sb[:], b2r.ap())
                ones_sb = wpool.tile([1, P], bf16, tag="ones")
                nc.vector.memset(ones_sb[:], 1.0)

            xgT_r = xgT.ap().rearrange("(c p) t -> p c t", p=P)

            for tb in range(nb):
                xg_sb = xpool.tile([P, NDC, TB], bf16, tag="xg")
                nc.sync.dma_start(
                    xg_sb[:], xgT_r[:, :, tb * TB:(tb + 1) * TB]
                )

                h_sb = hpool.tile([P, NFT * TB], bf16, tag="h")
                for ft in range(NFT):
                    ps = pp1.tile([P, TB], f32)
                    for c in range(NDC):
                        nc.tensor.matmul(
                            ps[:],
                            lhsT=w1_sb[c][:, ft * P:(ft + 1) * P],
                            rhs=xg_sb[:, c, :],
                            start=(c == 0),
                            stop=(c == NDC - 1),
                        )
                    nc.scalar.activation(
                        h_sb[:, ft * TB:(ft + 1) * TB], ps[:], Relu,
                        bias=b1_sb[:, ft:ft + 1],
                    )

                o_sb = opool.tile([P, NTS, D], bf16, tag="o")
                for s in range(NTS):
                    gcol = tb * NTS + s
                    for dh in range(NDH):
                        ps2 = pp2.tile([P, DH], f32)
                        for fc in range(NFT):
                            nc.tensor.matmul(
                                ps2[:],
                                lhsT=h_sb[:, fc * TB + s * P: fc * TB + s * P + P],
                                rhs=w2_sb[fc][:, dh * DH:(dh + 1) * DH],
                                start=(fc == 0),
                                stop=(fc == NFT - 1 and not has_b2),
                            )
                        if has_b2:
                            nc.tensor.matmul(
                                ps2[:],
                                lhsT=ones_sb[:, :],
                                rhs=b2_sb[:, dh * DH:(dh + 1) * DH],
                                start=False,
                                stop=True,
                            )
                        nc.scalar.mul(
                            o_sb[:, s, dh * DH:(dh + 1) * DH], ps2[:],
                            g_sb[:, gcol:gcol + 1],
                        )
                    nc.sync.dma_start(
                        out.ap()[tb * TB + s * P: tb * TB + (s + 1) * P, :],
                        o_sb[:, s, :],
                    )
    nc.compile()
    return nc


def build_nc_fp8(nb=16, tb=512):
    """fp8e4 DoubleRow variant: host pre-scales w1,w2 by 16 and gt by 1/16.
    Layouts: xgT8 [D,K] fp8 (k-blocks of 128 natural), w1dr [4,128,2,F] fp8,
    w2dr [16,128,2,D] fp8, b1t [128,32] f32, gt [128,K/128] f32 (pre-divided
    by 16). out [K,D] bf16."""
    import concourse.bacc as bacc
    import concourse.mybir as mybir
    import concourse.tile as tile

    fp8 = mybir.dt.float8e4
    bf16 = mybir.dt.bfloat16
    f32 = mybir.dt.float32
    Relu = mybir.ActivationFunctionType.Relu
    DR = mybir.MatmulPerfMode.DoubleRow

    k = nb * tb
    nts = tb // P           # token subtiles per block
    NC1 = D // 256          # 4 contraction chunks (mm1)
    NC2 = F // 256          # 16 contraction chunks (mm2)

    nc = bacc.Bacc("TRN2", target_bir_lowering=False, debug=False)
    xgT = nc.dram_tensor("xgT", [D, k], fp8, kind="ExternalInput")
    w1 = nc.dram_tensor("w1", [NC1, P, 2, F], fp8, kind="ExternalInput")
    w2 = nc.dram_tensor("w2", [NC2, P, 2, D], fp8, kind="ExternalInput")
    b1t = nc.dram_tensor("b1t", [P, NFT], f32, kind="ExternalInput")
    gt = nc.dram_tensor("gt", [P, k // P], f32, kind="ExternalInput")
    out = nc.dram_tensor("out", [k, D], bf16, kind="ExternalOutput")

    with tile.TileContext(nc) as tc:
        with (
            tc.tile_pool(name="wpool", bufs=1) as wpool,
            tc.tile_pool(name="xpool", bufs=3) as xpool,
            tc.tile_pool(name="hpool", bufs=2) as hpool,
            tc.tile_pool(name="opool", bufs=3) as opool,
            tc.tile_pool(name="pp1", bufs=4, space="PSUM") as pp1,
            tc.tile_pool(name="pp2", bufs=4, space="PSUM") as pp2,
        ):
            # Warm the ACT engine's function tables (Relu for mm1 evictions,
            # Copy for the gate-scale evictions) during the DMA head — the
            # first use of an activation function pays a ~2us table load that
            # otherwise backs up PSUM and stalls the PE early on.
            warm = wpool.tile([1, 1], f32, tag="warm")
            warm_b = wpool.tile([1, 1], f32, tag="warm_b")
            nc.vector.memset(warm[:], 0.0)
            nc.vector.memset(warm_b[:], 0.0)
            nc.scalar.activation(warm[:], warm[:], Relu, bias=warm_b[:, :1])
            nc.scalar.mul(warm[:], warm[:], 1.0)

            # xgT rows: chunk c covers D-rows [c*256, (c+1)*256); slot j holds
            # rows c*256 + j*128 + p  ->  "(c j p) t"
            xgT_r = xgT.ap().rearrange("(c j p) t -> p c j t", j=2, p=P)

            def load_xg(tbi):
                t = xpool.tile([P, NC1, 2, tb], fp8, tag="xg")
                nc.sync.dma_start(t[:], xgT_r[:, :, :, tbi * tb:(tbi + 1) * tb])
                return t

            # DMA issue order matters for the pipeline head (descriptors drain
            # in order per queue at ~320GB/s): sync queue carries xg0 then w1
            # (first matmul needs exactly these) then xg1, then w2 (only
            # needed ~27us in). The tiny b1/g (mm1 evictions need them early)
            # go on the gpsimd queue so they don't add head latency.
            b1_sb = wpool.tile([P, NFT], f32, tag="b1")
            nc.gpsimd.dma_start(b1_sb[:], b1t.ap())
            g_sb = wpool.tile([P, k // P], f32, tag="g")
            nc.gpsimd.dma_start(g_sb[:], gt.ap())
            xg_pre = [load_xg(0)]
            w1_sb = []
            for c in range(NC1):
                t = wpool.tile([P, 2, F], fp8, tag=f"w1_{c}")
                nc.sync.dma_start(t[:], w1.ap()[c])
                w1_sb.append(t)
            if nb > 1:
                xg_pre.append(load_xg(1))
            w2_sb = []
            for c in range(NC2):
                t = wpool.tile([P, 2, D], fp8, tag=f"w2_{c}")
                nc.sync.dma_start(t[:], w2.ap()[c])
                w2_sb.append(t)

            for tbi in range(nb):
                xg_sb = xg_pre[tbi] if tbi < len(xg_pre) else load_xg(tbi)

                h_sb = hpool.tile([P, NFT, tb], fp8, tag="h")
                for ft in range(NFT):
                    ps = pp1.tile([P, tb], f32)
                    for c in range(NC1):
                        nc.tensor.matmul(
                            ps[:],
                            lhsT=w1_sb[c][:, :, ft * P:(ft + 1) * P],
                            rhs=xg_sb[:, c, :, :],
                            start=(c == 0),
                            stop=(c == NC1 - 1),
                            perf_mode=DR,
                        )
                    # psum holds 16*(x@w1); relu((psum/16) + b1)
                    nc.scalar.activation(
                        h_sb[:, ft, :], ps[:], Relu,
                        bias=b1_sb[:, ft:ft + 1], scale=1.0 / 16.0,
                    )

                o_sb = opool.tile([P, nts, D], bf16, tag="o")
                for s in range(nts):
                    gcol = tbi * nts + s
                    for dh in range(NDH):
                        ps2 = pp2.tile([P, DH], f32)
                        for fc in range(NC2):
                            # lhsT: hT rows fc*256 + j*128 + p = F-tiles (2fc, 2fc+1)
                            nc.tensor.matmul(
                                ps2[:],
                                lhsT=h_sb[:, 2 * fc:2 * fc + 2, s * P:s * P + P],
                                rhs=w2_sb[fc][:, :, dh * DH:(dh + 1) * DH],
                                start=(fc == 0),
                                stop=(fc == NC2 - 1),
                                perf_mode=DR,
                            )
                        # psum holds 16*(h@w2); gt is pre-divided by 16
                        nc.scalar.mul(
                            o_sb[:, s, dh * DH:(dh + 1) * DH], ps2[:],
                            g_sb[:, gcol:gcol + 1],
                        )
                    nc.sync.dma_start(
                        out.ap()[tbi * tb + s * P: tbi * tb + (s + 1) * P, :],
                        o_sb[:, s, :],
                    )
    nc.compile()
    return nc


def _route_host(x, noise, w_route, b_route, w_noise, b_noise, top_k):
    """Replicates the oracle's router bit-exactly on CPU jax (op-for-op)."""
    import jax
    import jax.numpy as jnp

    cpu = jax.devices("cpu")[0]
    with jax.default_device(cpu):
        flat = jnp.asarray(np.asarray(x, np.float32)).reshape(-1, D)
        logits = (flat @ jnp.asarray(np.asarray(w_route, np.float32))
                  + jnp.asarray(np.asarray(b_route, np.float32))).T
        noise_logits = (flat @ jnp.asarray(np.asarray(w_noise, np.float32))
                        + jnp.asarray(np.asarray(b_noise, np.float32))).T
        noisy = logits + jnp.asarray(np.asarray(noise, np.float32)) * jax.nn.softplus(noise_logits)
        top_v, idx = jax.lax.top_k(noisy, top_k)
        gate = jax.nn.softmax(top_v, axis=-1)
        return np.asarray(idx), np.asarray(gate, np.float32)


def _gather_transpose(flat, idx):
    """[E,K] gather from flat [N,D] -> xgT [E, D, K] f32, via CPU jax."""
    import jax
    import jax.numpy as jnp

    cpu = jax.devices("cpu")[0]
    with jax.default_device(cpu):
        xg = jnp.take(jnp.asarray(flat), jnp.asarray(idx), axis=0)  # [E, K, D]
        xgT = jnp.transpose(xg, (0, 2, 1))
        return np.asarray(xgT)


def prepare(x, noise, w_route, b_route, w_noise, b_noise, w1, b1, w2, b2, top_k):
    """Host-side routing + sharding. Returns (build_key, in_maps, idx, flat)."""
    x = np.asarray(x, np.float32)
    w1 = np.asarray(w1, np.float32)
    b1 = np.asarray(b1, np.float32)
    w2 = np.asarray(w2, np.float32)
    b2 = np.asarray(b2, np.float32)
    assert int(top_k) == K

    idx, gate = _route_host(x, noise, w_route, b_route, w_noise, b_noise, int(top_k))
    flat = x.reshape(-1, D)
    xgT = _gather_transpose(flat, idx)

    has_b2 = bool(np.any(b2))
    in_maps = []
    if not has_b2:
        # fp8e4 DoubleRow path (w1,w2 pre-scaled by 16; gate divided by 16)
        key = "fp8"
        f8 = ml_dtypes.float8_e4m3

        def to_f8(a):
            # clip to TRN fp8e4's +-240 range so outliers saturate, not inf
            return np.clip(a, -240.0, 240.0).astype(f8)

        for e in range(E):
            in_maps.append({
                "xgT": to_f8(xgT[e]),
                "w1": np.ascontiguousarray(
                    to_f8(w1[e] * 16).reshape(4, 2, P, F).transpose(0, 2, 1, 3)),
                "w2": np.ascontiguousarray(
                    to_f8(w2[e] * 16).reshape(16, 2, P, D).transpose(0, 2, 1, 3)),
                "b1t": np.ascontiguousarray(b1[e].reshape(NFT, P).T.astype(np.float32)),
                "gt": np.ascontiguousarray(
                    (gate[e] / 16.0).reshape(K // P, P).T.astype(np.float32)),
            })
    else:
        key = "bf16_b2"
        bf = ml_dtypes.bfloat16
        for e in range(E):
            m = {
                "xgT": xgT[e].astype(bf),
                "w1": w1[e].astype(bf),
                "w2": w2[e].astype(bf),
                "b1t": np.ascontiguousarray(b1[e].reshape(NFT, P).T.astype(np.float32)),
                "gt": np.ascontiguousarray(gate[e].reshape(K // P, P).T.astype(np.float32)),
            }
            m["b2r"] = b2[e].reshape(1, D).astype(bf)
            in_maps.append(m)

    return key, in_maps, idx, flat


def build_for(key):
    if key not in _STATE:
        _STATE[key] = build_nc_fp8(nb=16, tb=512) if key == "fp8" else build_nc(True)
    return _STATE[key]


def _run_device_subprocess(key, in_maps):
    """Disaster-recovery path: a device execution failure poisons the PJRT
    client for the rest of the process, but a fresh process's first
    execution recovers. Ship the per-core inputs to a new interpreter."""
    import os
    import subprocess
    import sys
    import tempfile

    tmp = tempfile.mkdtemp()
    inp, outp = os.path.join(tmp, "in.npz"), os.path.join(tmp, "out.npz")
    save = {}
    for e, m in enumerate(in_maps):
        for name, arr in m.items():
            dt = str(arr.dtype)
            save[f"{e}|{name}|{dt}"] = (
                arr if arr.dtype == np.float32 else arr.view(np.uint8))
    np.savez(inp, **save)
    kdir = os.path.dirname(os.path.abspath(__file__))
    runner = f"""
import numpy as np, ml_dtypes, sys
sys.path.insert(0, {kdir!r})
import kernel as kmod
from concourse.bass_utils import run_bass_kernel_spmd
z = np.load({inp!r})
in_maps = [dict() for _ in range(kmod.E)]
for kk in z.files:
    e, name, dt = kk.split('|')
    a = z[kk]
    if dt != 'float32':
        a = a.view(getattr(ml_dtypes, dt))
    in_maps[int(e)][name] = a
nc = kmod.build_for({key!r})
res = run_bass_kernel_spmd(nc, in_maps, core_ids=list(range(kmod.E)))
np.savez({outp!r}, **{{str(e): np.asarray(res.results[e]['out']).view(np.uint8)
                      for e in range(kmod.E)}})
"""
    subprocess.run([sys.executable, "-c", runner], check=True)
    z = np.load(outp)
    return [{"out": z[str(e)].view(ml_dtypes.bfloat16)} for e in range(E)]


def run_device(key, in_maps):
    from concourse.bass_utils import run_bass_kernel_spmd

    nc = build_for(key)
    for _ in range(2):
        try:
            return run_bass_kernel_spmd(nc, in_maps, core_ids=list(range(E))).results
        except Exception:
            pass
    return _run_device_subprocess(key, in_maps)


def kernel(x, noise, w_route, b_route, w_noise, b_noise, w1, b1, w2, b2, top_k):
    x = np.asarray(x, np.float32)
    B, S, _ = x.shape
    key, in_maps, idx, flat = prepare(
        x, noise, w_route, b_route, w_noise, b_noise, w1, b1, w2, b2, top_k)

    results = run_device(key, in_maps)

    final = flat.copy()
    for e in range(E):
        final[idx[e]] += np.asarray(results[e]["out"], dtype=np.float32)
    return final.reshape(B, S, D), idx


# revision 36
# speedup vs baseline: 1.0011x; 1.0011x over previous
"""Expert-choice token-sparse MoE for Trainium2 (8 NeuronCores, expert-parallel).

Contract: kernel(**inputs) takes the FULL unsharded inputs and returns the FULL
output, matching reference.reference(): a tuple (final [B,S,D] f32, idx [E,K] i32).

Sharding strategy (expert-parallel, per the hint):
  - The router + top-k runs on host CPU (bit-exact replication of the oracle's
    jax ops — idx is an integer output that must match exactly, which requires
    bit-identical fp32 routing scores; the heavy compute is NOT here).
  - Expert e's gathered tokens + weights are shipped to core e ("all-to-all
    dispatch" done host-side as part of sharding).
  - Each core runs the dense MLP for its 8192 selected tokens on the Tensor
    engine in fp8e4 DoubleRow (2 fp8 MACs/cell/cycle):
    hT = relu(w1^T @ xgT + b1); out = (hT^T @ w2) * gate  (~137 GFLOP/core).
    The fp8 quantization error is attenuated by the gate (~1e-4) relative to
    the fp32 residual, so final relative error stays ~2e-5. A bf16 variant
    handles the (spec-wise impossible) b2 != 0 case.
  - Host combines: scatter-add each expert's compact [K, D] output + residual.
Measured: ~911 us HW exec per core (92% MFU; fp8 roofline for the 137 GFLOP
is ~874 us + ~14 us DMA head + ~12 us drain tail).
"""

import numpy as np
import ml_dtypes

E = 8
D = 1024
F = 4096
N = 32768
K = 8192
P = 128
TB = 256              # tokens per device block
NDC = D // P          # 8 d-chunks
NFT = F // P          # 32 f-tiles
NB = K // TB          # 32 token blocks
NTS = TB // P         # 2 token subtiles per block
DH = 512              # out D half (PSUM bank)
NDH = D // DH         # 2

_STATE = {}


def build_nc(has_b2=False, nb=NB):
    import concourse.bacc as bacc
    import concourse.mybir as mybir
    import concourse.tile as tile

    bf16 = mybir.dt.bfloat16
    f32 = mybir.dt.float32
    Relu = mybir.ActivationFunctionType.Relu

    k = nb * TB
    nc = bacc.Bacc("TRN2", target_bir_lowering=False, debug=False)
    xgT = nc.dram_tensor("xgT", [D, k], bf16, kind="ExternalInput")
    w1 = nc.dram_tensor("w1", [D, F], bf16, kind="ExternalInput")
    w2 = nc.dram_tensor("w2", [F, D], bf16, kind="ExternalInput")
    b1t = nc.dram_tensor("b1t", [P, NFT], f32, kind="ExternalInput")
    gt = nc.dram_tensor("gt", [P, k // P], f32, kind="ExternalInput")
    if has_b2:
        b2r = nc.dram_tensor("b2r", [1, D], bf16, kind="ExternalInput")
    out = nc.dram_tensor("out", [k, D], bf16, kind="ExternalOutput")

    with tile.TileContext(nc) as tc:
        with (
            tc.tile_pool(name="wpool", bufs=1) as wpool,
            tc.tile_pool(name="xpool", bufs=3) as xpool,
            tc.tile_pool(name="hpool", bufs=2) as hpool,
            tc.tile_pool(name="opool", bufs=3) as opool,
            tc.tile_pool(name="pp1", bufs=4, space="PSUM") as pp1,
            tc.tile_pool(name="pp2", bufs=4, space="PSUM") as pp2,
        ):
            # resident weights
            w1_sb = []
            for c in range(NDC):
                t = wpool.tile([P, F], bf16, tag=f"w1_{c}")
                nc.sync.dma_start(t[:], w1.ap()[c * P:(c + 1) * P, :])
                w1_sb.append(t)
            w2_sb = []
            for c in range(NFT):
                t = wpool.tile([P, D], bf16, tag=f"w2_{c}")
                nc.sync.dma_start(t[:], w2.ap()[c * P:(c + 1) * P, :])
                w2_sb.append(t)
            b1_sb = wpool.tile([P, NFT], f32, tag="b1")
            nc.sync.dma_start(b1_sb[:], b1t.ap())
            g_sb = wpool.tile([P, k // P], f32, tag="g")
            nc.sync.dma_start(g_sb[:], gt.ap())
            if has_b2:
                b2_sb = wpool.tile([1, D], bf16, tag="b2")
                nc.sync.dma_start(b2_sb[:], b2r.ap())
                ones_sb = wpool.tile([1, P], bf16, tag="ones")
                nc.vector.memset(ones_sb[:], 1.0)

            xgT_r = xgT.ap().rearrange("(c p) t -> p c t", p=P)

            for tb in range(nb):
                xg_sb = xpool.tile([P, NDC, TB], bf16, tag="xg")
                nc.sync.dma_start(
                    xg_sb[:], xgT_r[:, :, tb * TB:(tb + 1) * TB]
                )

                h_sb = hpool.tile([P, NFT * TB], bf16, tag="h")
                for ft in range(NFT):
                    ps = pp1.tile([P, TB], f32)
                    for c in range(NDC):
                        nc.tensor.matmul(
                            ps[:],
                            lhsT=w1_sb[c][:, ft * P:(ft + 1) * P],
                            rhs=xg_sb[:, c, :],
                            start=(c == 0),
                            stop=(c == NDC - 1),
                        )
                    nc.scalar.activation(
                        h_sb[:, ft * TB:(ft + 1) * TB], ps[:], Relu,
                        bias=b1_sb[:, ft:ft + 1],
                    )

                o_sb = opool.tile([P, NTS, D], bf16, tag="o")
                for s in range(NTS):
                    gcol = tb * NTS + s
                    for dh in range(NDH):
                        ps2 = pp2.tile([P, DH], f32)
                        for fc in range(NFT):
                            nc.tensor.matmul(
                                ps2[:],
                                lhsT=h_sb[:, fc * TB + s * P: fc * TB + s * P + P],
                                rhs=w2_sb[fc][:, dh * DH:(dh + 1) * DH],
                                start=(fc == 0),
                                stop=(fc == NFT - 1 and not has_b2),
                            )
                        if has_b2:
                            nc.tensor.matmul(
                                ps2[:],
                                lhsT=ones_sb[:, :],
                                rhs=b2_sb[:, dh * DH:(dh + 1) * DH],
                                start=False,
                                stop=True,
                            )
                        nc.scalar.mul(
                            o_sb[:, s, dh * DH:(dh + 1) * DH], ps2[:],
                            g_sb[:, gcol:gcol + 1],
                        )
                    nc.sync.dma_start(
                        out.ap()[tb * TB + s * P: tb * TB + (s + 1) * P, :],
                        o_sb[:, s, :],
                    )
    nc.compile()
    return nc


def build_nc_fp8(nb=16, tb=512, has_b1=False):
    """fp8e4 DoubleRow variant: host pre-scales w1,w2 by 16 and gt by 1/16.
    Layouts: xgT8 [D,K] fp8 (k-blocks of 128 natural), w1dr [4,128,2,F] fp8,
    w2dr [16,128,2,D] fp8, b1t [128,32] f32 (only if has_b1), gt [128,K/128]
    f32 (pre-divided by 16). out [K,D] bf16. When b1 is all-zero (the spec
    case) the relu evictions use a float-const bias, removing the b1 DMA
    from the eviction dependency chain."""
    import concourse.bacc as bacc
    import concourse.mybir as mybir
    import concourse.tile as tile

    fp8 = mybir.dt.float8e4
    bf16 = mybir.dt.bfloat16
    f32 = mybir.dt.float32
    Relu = mybir.ActivationFunctionType.Relu
    DR = mybir.MatmulPerfMode.DoubleRow

    k = nb * tb
    nts = tb // P           # token subtiles per block
    NC1 = D // 256          # 4 contraction chunks (mm1)
    NC2 = F // 256          # 16 contraction chunks (mm2)

    nc = bacc.Bacc("TRN2", target_bir_lowering=False, debug=False)
    xgT = nc.dram_tensor("xgT", [D, k], fp8, kind="ExternalInput")
    w1 = nc.dram_tensor("w1", [NC1, P, 2, F], fp8, kind="ExternalInput")
    w2 = nc.dram_tensor("w2", [NC2, P, 2, D], fp8, kind="ExternalInput")
    if has_b1:
        b1t = nc.dram_tensor("b1t", [P, NFT], f32, kind="ExternalInput")
    gt = nc.dram_tensor("gt", [P, k // P], f32, kind="ExternalInput")
    out = nc.dram_tensor("out", [k, D], bf16, kind="ExternalOutput")

    with tile.TileContext(nc) as tc:
        with (
            tc.tile_pool(name="wpool", bufs=1) as wpool,
            tc.tile_pool(name="xpool", bufs=3) as xpool,
            tc.tile_pool(name="hpool", bufs=2) as hpool,
            tc.tile_pool(name="opool", bufs=3) as opool,
            tc.tile_pool(name="pp1", bufs=4, space="PSUM") as pp1,
            tc.tile_pool(name="pp2", bufs=4, space="PSUM") as pp2,
        ):
            # Warm the ACT engine's function tables (Relu for mm1 evictions,
            # Copy for the gate-scale evictions) during the DMA head — the
            # first use of an activation function pays a ~2us table load that
            # otherwise backs up PSUM and stalls the PE early on.
            warm = wpool.tile([1, 1], f32, tag="warm")
            warm_b = wpool.tile([1, 1], f32, tag="warm_b")
            nc.vector.memset(warm[:], 0.0)
            nc.vector.memset(warm_b[:], 0.0)
            nc.scalar.activation(warm[:], warm[:], Relu, bias=warm_b[:, :1])
            nc.scalar.mul(warm[:], warm[:], 1.0)

            # xgT rows: chunk c covers D-rows [c*256, (c+1)*256); slot j holds
            # rows c*256 + j*128 + p  ->  "(c j p) t"
            xgT_r = xgT.ap().rearrange("(c j p) t -> p c j t", j=2, p=P)

            def load_xg(tbi):
                t = xpool.tile([P, NC1, 2, tb], fp8, tag="xg")
                nc.sync.dma_start(t[:], xgT_r[:, :, :, tbi * tb:(tbi + 1) * tb])
                return t

            # DMA issue order matters for the pipeline head (descriptors drain
            # in order per queue at ~320GB/s): sync queue carries xg0 then w1
            # (first matmul needs exactly these) then xg1, then w2 (only
            # needed ~27us in). The tiny b1/g (mm1 evictions need them early)
            # go on the gpsimd queue so they don't add head latency.
            if has_b1:
                b1_sb = wpool.tile([P, NFT], f32, tag="b1")
                nc.gpsimd.dma_start(b1_sb[:], b1t.ap())
            g_sb = wpool.tile([P, k // P], f32, tag="g")
            nc.gpsimd.dma_start(g_sb[:], gt.ap())
            xg_pre = [load_xg(0)]
            w1_sb = []
            for c in range(NC1):
                t = wpool.tile([P, 2, F], fp8, tag=f"w1_{c}")
                nc.sync.dma_start(t[:], w1.ap()[c])
                w1_sb.append(t)
            if nb > 1:
                xg_pre.append(load_xg(1))
            w2_sb = []
            for c in range(NC2):
                t = wpool.tile([P, 2, D], fp8, tag=f"w2_{c}")
                nc.sync.dma_start(t[:], w2.ap()[c])
                w2_sb.append(t)

            for tbi in range(nb):
                xg_sb = xg_pre[tbi] if tbi < len(xg_pre) else load_xg(tbi)

                h_sb = hpool.tile([P, NFT, tb], fp8, tag="h")
                for ft in range(NFT):
                    ps = pp1.tile([P, tb], f32)
                    for c in range(NC1):
                        nc.tensor.matmul(
                            ps[:],
                            lhsT=w1_sb[c][:, :, ft * P:(ft + 1) * P],
                            rhs=xg_sb[:, c, :, :],
                            start=(c == 0),
                            stop=(c == NC1 - 1),
                            perf_mode=DR,
                        )
                    # psum holds 16*(x@w1); relu((psum/16) + b1)
                    nc.scalar.activation(
                        h_sb[:, ft, :], ps[:], Relu,
                        bias=(b1_sb[:, ft:ft + 1] if has_b1 else 0.0),
                        scale=1.0 / 16.0,
                    )

                o_sb = opool.tile([P, nts, D], bf16, tag="o")
                for s in range(nts):
                    gcol = tbi * nts + s
                    for dh in range(NDH):
                        ps2 = pp2.tile([P, DH], f32)
                        for fc in range(NC2):
                            # lhsT: hT rows fc*256 + j*128 + p = F-tiles (2fc, 2fc+1)
                            nc.tensor.matmul(
                                ps2[:],
                                lhsT=h_sb[:, 2 * fc:2 * fc + 2, s * P:s * P + P],
                                rhs=w2_sb[fc][:, :, dh * DH:(dh + 1) * DH],
                                start=(fc == 0),
                                stop=(fc == NC2 - 1),
                                perf_mode=DR,
                            )
                        # psum holds 16*(h@w2); gt is pre-divided by 16
                        nc.scalar.mul(
                            o_sb[:, s, dh * DH:(dh + 1) * DH], ps2[:],
                            g_sb[:, gcol:gcol + 1],
                        )
                    nc.sync.dma_start(
                        out.ap()[tbi * tb + s * P: tbi * tb + (s + 1) * P, :],
                        o_sb[:, s, :],
                    )
    nc.compile()
    return nc


def _route_host(x, noise, w_route, b_route, w_noise, b_noise, top_k):
    """Replicates the oracle's router bit-exactly on CPU jax (op-for-op)."""
    import jax
    import jax.numpy as jnp

    cpu = jax.devices("cpu")[0]
    with jax.default_device(cpu):
        flat = jnp.asarray(np.asarray(x, np.float32)).reshape(-1, D)
        logits = (flat @ jnp.asarray(np.asarray(w_route, np.float32))
                  + jnp.asarray(np.asarray(b_route, np.float32))).T
        noise_logits = (flat @ jnp.asarray(np.asarray(w_noise, np.float32))
                        + jnp.asarray(np.asarray(b_noise, np.float32))).T
        noisy = logits + jnp.asarray(np.asarray(noise, np.float32)) * jax.nn.softplus(noise_logits)
        top_v, idx = jax.lax.top_k(noisy, top_k)
        gate = jax.nn.softmax(top_v, axis=-1)
        return np.asarray(idx), np.asarray(gate, np.float32)


def _gather_transpose(flat, idx):
    """[E,K] gather from flat [N,D] -> xgT [E, D, K] f32, via CPU jax."""
    import jax
    import jax.numpy as jnp

    cpu = jax.devices("cpu")[0]
    with jax.default_device(cpu):
        xg = jnp.take(jnp.asarray(flat), jnp.asarray(idx), axis=0)  # [E, K, D]
        xgT = jnp.transpose(xg, (0, 2, 1))
        return np.asarray(xgT)


def prepare(x, noise, w_route, b_route, w_noise, b_noise, w1, b1, w2, b2, top_k):
    """Host-side routing + sharding. Returns (build_key, in_maps, idx, flat)."""
    x = np.asarray(x, np.float32)
    w1 = np.asarray(w1, np.float32)
    b1 = np.asarray(b1, np.float32)
    w2 = np.asarray(w2, np.float32)
    b2 = np.asarray(b2, np.float32)
    assert int(top_k) == K

    idx, gate = _route_host(x, noise, w_route, b_route, w_noise, b_noise, int(top_k))
    flat = x.reshape(-1, D)
    xgT = _gather_transpose(flat, idx)

    has_b2 = bool(np.any(b2))
    has_b1 = bool(np.any(b1))
    in_maps = []
    if not has_b2:
        # fp8e4 DoubleRow path (w1,w2 pre-scaled by 16; gate divided by 16)
        key = "fp8_b1" if has_b1 else "fp8"
        f8 = ml_dtypes.float8_e4m3

        def to_f8(a):
            # clip to TRN fp8e4's +-240 range so outliers saturate, not inf
            return np.clip(a, -240.0, 240.0).astype(f8)

        for e in range(E):
            m = {
                "xgT": to_f8(xgT[e]),
                "w1": np.ascontiguousarray(
                    to_f8(w1[e] * 16).reshape(4, 2, P, F).transpose(0, 2, 1, 3)),
                "w2": np.ascontiguousarray(
                    to_f8(w2[e] * 16).reshape(16, 2, P, D).transpose(0, 2, 1, 3)),
                "gt": np.ascontiguousarray(
                    (gate[e] / 16.0).reshape(K // P, P).T.astype(np.float32)),
            }
            if has_b1:
                m["b1t"] = np.ascontiguousarray(
                    b1[e].reshape(NFT, P).T.astype(np.float32))
            in_maps.append(m)
    else:
        key = "bf16_b2"
        bf = ml_dtypes.bfloat16
        for e in range(E):
            m = {
                "xgT": xgT[e].astype(bf),
                "w1": w1[e].astype(bf),
                "w2": w2[e].astype(bf),
                "b1t": np.ascontiguousarray(b1[e].reshape(NFT, P).T.astype(np.float32)),
                "gt": np.ascontiguousarray(gate[e].reshape(K // P, P).T.astype(np.float32)),
            }
            m["b2r"] = b2[e].reshape(1, D).astype(bf)
            in_maps.append(m)

    return key, in_maps, idx, flat


def build_for(key):
    if key not in _STATE:
        if key == "fp8":
            _STATE[key] = build_nc_fp8(nb=16, tb=512, has_b1=False)
        elif key == "fp8_b1":
            _STATE[key] = build_nc_fp8(nb=16, tb=512, has_b1=True)
        else:
            _STATE[key] = build_nc(True)
    return _STATE[key]


def _run_device_subprocess(key, in_maps):
    """Disaster-recovery path: a device execution failure poisons the PJRT
    client for the rest of the process, but a fresh process's first
    execution recovers. Ship the per-core inputs to a new interpreter."""
    import os
    import subprocess
    import sys
    import tempfile

    tmp = tempfile.mkdtemp()
    inp, outp = os.path.join(tmp, "in.npz"), os.path.join(tmp, "out.npz")
    save = {}
    for e, m in enumerate(in_maps):
        for name, arr in m.items():
            dt = str(arr.dtype)
            save[f"{e}|{name}|{dt}"] = (
                arr if arr.dtype == np.float32 else arr.view(np.uint8))
    np.savez(inp, **save)
    kdir = os.path.dirname(os.path.abspath(__file__))
    runner = f"""
import numpy as np, ml_dtypes, sys
sys.path.insert(0, {kdir!r})
import kernel as kmod
from concourse.bass_utils import run_bass_kernel_spmd
z = np.load({inp!r})
in_maps = [dict() for _ in range(kmod.E)]
for kk in z.files:
    e, name, dt = kk.split('|')
    a = z[kk]
    if dt != 'float32':
        a = a.view(getattr(ml_dtypes, dt))
    in_maps[int(e)][name] = a
nc = kmod.build_for({key!r})
res = run_bass_kernel_spmd(nc, in_maps, core_ids=list(range(kmod.E)))
np.savez({outp!r}, **{{str(e): np.asarray(res.results[e]['out']).view(np.uint8)
                      for e in range(kmod.E)}})
"""
    subprocess.run([sys.executable, "-c", runner], check=True)
    z = np.load(outp)
    return [{"out": z[str(e)].view(ml_dtypes.bfloat16)} for e in range(E)]


def run_device(key, in_maps):
    from concourse.bass_utils import run_bass_kernel_spmd

    nc = build_for(key)
    for _ in range(2):
        try:
            return run_bass_kernel_spmd(nc, in_maps, core_ids=list(range(E))).results
        except Exception:
            pass
    return _run_device_subprocess(key, in_maps)


def kernel(x, noise, w_route, b_route, w_noise, b_noise, w1, b1, w2, b2, top_k):
    x = np.asarray(x, np.float32)
    B, S, _ = x.shape
    key, in_maps, idx, flat = prepare(
        x, noise, w_route, b_route, w_noise, b_noise, w1, b1, w2, b2, top_k)

    results = run_device(key, in_maps)

    final = flat.copy()
    for e in range(E):
        final[idx[e]] += np.asarray(results[e]["out"], dtype=np.float32)
    return final.reshape(B, S, D), idx


# revision 37
# speedup vs baseline: 1.0027x; 1.0017x over previous
"""Expert-choice token-sparse MoE for Trainium2 (8 NeuronCores, expert-parallel).

Contract: kernel(**inputs) takes the FULL unsharded inputs and returns the FULL
output, matching reference.reference(): a tuple (final [B,S,D] f32, idx [E,K] i32).

Sharding strategy (expert-parallel, per the hint):
  - The router + top-k runs on host CPU (bit-exact replication of the oracle's
    jax ops — idx is an integer output that must match exactly, which requires
    bit-identical fp32 routing scores; the heavy compute is NOT here).
  - Expert e's gathered tokens + weights are shipped to core e ("all-to-all
    dispatch" done host-side as part of sharding).
  - Each core runs the dense MLP for its 8192 selected tokens on the Tensor
    engine in fp8e4 DoubleRow (2 fp8 MACs/cell/cycle):
    hT = relu(w1^T @ xgT + b1); out = (hT^T @ w2) * gate  (~137 GFLOP/core).
    The fp8 quantization error is attenuated by the gate (~1e-4) relative to
    the fp32 residual, so final relative error stays ~2e-5. A bf16 variant
    handles the (spec-wise impossible) b2 != 0 case.
  - Host combines: scatter-add each expert's compact [K, D] output + residual.
Measured: ~911 us HW exec per core (92% MFU; fp8 roofline for the 137 GFLOP
is ~874 us + ~14 us DMA head + ~12 us drain tail).
"""

import numpy as np
import ml_dtypes

E = 8
D = 1024
F = 4096
N = 32768
K = 8192
P = 128
TB = 256              # tokens per device block
NDC = D // P          # 8 d-chunks
NFT = F // P          # 32 f-tiles
NB = K // TB          # 32 token blocks
NTS = TB // P         # 2 token subtiles per block
DH = 512              # out D half (PSUM bank)
NDH = D // DH         # 2

_STATE = {}


def build_nc(has_b2=False, nb=NB):
    import concourse.bacc as bacc
    import concourse.mybir as mybir
    import concourse.tile as tile

    bf16 = mybir.dt.bfloat16
    f32 = mybir.dt.float32
    Relu = mybir.ActivationFunctionType.Relu

    k = nb * TB
    nc = bacc.Bacc("TRN2", target_bir_lowering=False, debug=False)
    xgT = nc.dram_tensor("xgT", [D, k], bf16, kind="ExternalInput")
    w1 = nc.dram_tensor("w1", [D, F], bf16, kind="ExternalInput")
    w2 = nc.dram_tensor("w2", [F, D], bf16, kind="ExternalInput")
    b1t = nc.dram_tensor("b1t", [P, NFT], f32, kind="ExternalInput")
    gt = nc.dram_tensor("gt", [P, k // P], f32, kind="ExternalInput")
    if has_b2:
        b2r = nc.dram_tensor("b2r", [1, D], bf16, kind="ExternalInput")
    out = nc.dram_tensor("out", [k, D], bf16, kind="ExternalOutput")

    with tile.TileContext(nc) as tc:
        with (
            tc.tile_pool(name="wpool", bufs=1) as wpool,
            tc.tile_pool(name="xpool", bufs=3) as xpool,
            tc.tile_pool(name="hpool", bufs=2) as hpool,
            tc.tile_pool(name="opool", bufs=3) as opool,
            tc.tile_pool(name="pp1", bufs=4, space="PSUM") as pp1,
            tc.tile_pool(name="pp2", bufs=4, space="PSUM") as pp2,
        ):
            # resident weights
            w1_sb = []
            for c in range(NDC):
                t = wpool.tile([P, F], bf16, tag=f"w1_{c}")
                nc.sync.dma_start(t[:], w1.ap()[c * P:(c + 1) * P, :])
                w1_sb.append(t)
            w2_sb = []
            for c in range(NFT):
                t = wpool.tile([P, D], bf16, tag=f"w2_{c}")
                nc.sync.dma_start(t[:], w2.ap()[c * P:(c + 1) * P, :])
                w2_sb.append(t)
            b1_sb = wpool.tile([P, NFT], f32, tag="b1")
            nc.sync.dma_start(b1_sb[:], b1t.ap())
            g_sb = wpool.tile([P, k // P], f32, tag="g")
            nc.sync.dma_start(g_sb[:], gt.ap())
            if has_b2:
                b2_sb = wpool.tile([1, D], bf16, tag="b2")
                nc.sync.dma_start(b2_sb[:], b2r.ap())
                ones_sb = wpool.tile([1, P], bf16, tag="ones")
                nc.vector.memset(ones_sb[:], 1.0)

            xgT_r = xgT.ap().rearrange("(c p) t -> p c t", p=P)

            for tb in range(nb):
                xg_sb = xpool.tile([P, NDC, TB], bf16, tag="xg")
                nc.sync.dma_start(
                    xg_sb[:], xgT_r[:, :, tb * TB:(tb + 1) * TB]
                )

                h_sb = hpool.tile([P, NFT * TB], bf16, tag="h")
                for ft in range(NFT):
                    ps = pp1.tile([P, TB], f32)
                    for c in range(NDC):
                        nc.tensor.matmul(
                            ps[:],
                            lhsT=w1_sb[c][:, ft * P:(ft + 1) * P],
                            rhs=xg_sb[:, c, :],
                            start=(c == 0),
                            stop=(c == NDC - 1),
                        )
                    nc.scalar.activation(
                        h_sb[:, ft * TB:(ft + 1) * TB], ps[:], Relu,
                        bias=b1_sb[:, ft:ft + 1],
                    )

                o_sb = opool.tile([P, NTS, D], bf16, tag="o")
                for s in range(NTS):
                    gcol = tb * NTS + s
                    for dh in range(NDH):
                        ps2 = pp2.tile([P, DH], f32)
                        for fc in range(NFT):
                            nc.tensor.matmul(
                                ps2[:],
                                lhsT=h_sb[:, fc * TB + s * P: fc * TB + s * P + P],
                                rhs=w2_sb[fc][:, dh * DH:(dh + 1) * DH],
                                start=(fc == 0),
                                stop=(fc == NFT - 1 and not has_b2),
                            )
                        if has_b2:
                            nc.tensor.matmul(
                                ps2[:],
                                lhsT=ones_sb[:, :],
                                rhs=b2_sb[:, dh * DH:(dh + 1) * DH],
                                start=False,
                                stop=True,
                            )
                        nc.scalar.mul(
                            o_sb[:, s, dh * DH:(dh + 1) * DH], ps2[:],
                            g_sb[:, gcol:gcol + 1],
                        )
                    nc.sync.dma_start(
                        out.ap()[tb * TB + s * P: tb * TB + (s + 1) * P, :],
                        o_sb[:, s, :],
                    )
    nc.compile()
    return nc


def build_nc_fp8(nb=16, tb=512, has_b1=False):
    """fp8e4 DoubleRow variant: host pre-scales w1,w2 by 16 and gt by 1/16.
    Layouts: xgT8 [D,K] fp8 (k-blocks of 128 natural), w1dr [4,128,2,F] fp8,
    w2dr [16,128,2,D] fp8, b1t [128,32] f32 (only if has_b1), gt [128,K/128]
    f32 (pre-divided by 16). out [K,D] bf16. When b1 is all-zero (the spec
    case) the relu evictions use a float-const bias, removing the b1 DMA
    from the eviction dependency chain."""
    import concourse.bacc as bacc
    import concourse.mybir as mybir
    import concourse.tile as tile

    fp8 = mybir.dt.float8e4
    bf16 = mybir.dt.bfloat16
    f32 = mybir.dt.float32
    Relu = mybir.ActivationFunctionType.Relu
    DR = mybir.MatmulPerfMode.DoubleRow

    k = nb * tb
    nts = tb // P           # token subtiles per block
    NC1 = D // 256          # 4 contraction chunks (mm1)
    NC2 = F // 256          # 16 contraction chunks (mm2)

    nc = bacc.Bacc("TRN2", target_bir_lowering=False, debug=False)
    xgT = nc.dram_tensor("xgT", [D, k], fp8, kind="ExternalInput")
    w1 = nc.dram_tensor("w1", [NC1, P, 2, F], fp8, kind="ExternalInput")
    w2 = nc.dram_tensor("w2", [NC2, P, 2, D], fp8, kind="ExternalInput")
    if has_b1:
        b1t = nc.dram_tensor("b1t", [P, NFT], f32, kind="ExternalInput")
    gt = nc.dram_tensor("gt", [P, k // P], f32, kind="ExternalInput")
    out = nc.dram_tensor("out", [k, D], bf16, kind="ExternalOutput")

    with tile.TileContext(nc) as tc:
        with (
            tc.tile_pool(name="wpool", bufs=1) as wpool,
            tc.tile_pool(name="xpool", bufs=3) as xpool,
            tc.tile_pool(name="hpool", bufs=2) as hpool,
            tc.tile_pool(name="opool", bufs=3) as opool,
            tc.tile_pool(name="pp1", bufs=6, space="PSUM") as pp1,
            tc.tile_pool(name="pp2", bufs=2, space="PSUM") as pp2,
        ):
            # Warm the ACT engine's function tables (Relu for mm1 evictions,
            # Copy for the gate-scale evictions) during the DMA head — the
            # first use of an activation function pays a ~2us table load that
            # otherwise backs up PSUM and stalls the PE early on.
            warm = wpool.tile([1, 1], f32, tag="warm")
            warm_b = wpool.tile([1, 1], f32, tag="warm_b")
            nc.vector.memset(warm[:], 0.0)
            nc.vector.memset(warm_b[:], 0.0)
            nc.scalar.activation(warm[:], warm[:], Relu, bias=warm_b[:, :1])
            nc.scalar.mul(warm[:], warm[:], 1.0)

            # xgT rows: chunk c covers D-rows [c*256, (c+1)*256); slot j holds
            # rows c*256 + j*128 + p  ->  "(c j p) t"
            xgT_r = xgT.ap().rearrange("(c j p) t -> p c j t", j=2, p=P)

            def load_xg(tbi):
                t = xpool.tile([P, NC1, 2, tb], fp8, tag="xg")
                nc.sync.dma_start(t[:], xgT_r[:, :, :, tbi * tb:(tbi + 1) * tb])
                return t

            # DMA issue order matters for the pipeline head (descriptors drain
            # in order per queue at ~320GB/s): sync queue carries xg0 then w1
            # (first matmul needs exactly these) then xg1, then w2 (only
            # needed ~27us in). The tiny b1/g (mm1 evictions need them early)
            # go on the gpsimd queue so they don't add head latency.
            if has_b1:
                b1_sb = wpool.tile([P, NFT], f32, tag="b1")
                nc.gpsimd.dma_start(b1_sb[:], b1t.ap())
            g_sb = wpool.tile([P, k // P], f32, tag="g")
            nc.gpsimd.dma_start(g_sb[:], gt.ap())
            xg_pre = [load_xg(0)]
            w1_sb = []
            for c in range(NC1):
                t = wpool.tile([P, 2, F], fp8, tag=f"w1_{c}")
                nc.sync.dma_start(t[:], w1.ap()[c])
                w1_sb.append(t)
            if nb > 1:
                xg_pre.append(load_xg(1))
            w2_sb = []
            for c in range(NC2):
                t = wpool.tile([P, 2, D], fp8, tag=f"w2_{c}")
                nc.sync.dma_start(t[:], w2.ap()[c])
                w2_sb.append(t)

            for tbi in range(nb):
                xg_sb = xg_pre[tbi] if tbi < len(xg_pre) else load_xg(tbi)

                h_sb = hpool.tile([P, NFT, tb], fp8, tag="h")
                for ft in range(NFT):
                    ps = pp1.tile([P, tb], f32)
                    for c in range(NC1):
                        nc.tensor.matmul(
                            ps[:],
                            lhsT=w1_sb[c][:, :, ft * P:(ft + 1) * P],
                            rhs=xg_sb[:, c, :, :],
                            start=(c == 0),
                            stop=(c == NC1 - 1),
                            perf_mode=DR,
                        )
                    # psum holds 16*(x@w1); relu((psum/16) + b1)
                    nc.scalar.activation(
                        h_sb[:, ft, :], ps[:], Relu,
                        bias=(b1_sb[:, ft:ft + 1] if has_b1 else 0.0),
                        scale=1.0 / 16.0,
                    )

                o_sb = opool.tile([P, nts, D], bf16, tag="o")
                for s in range(nts):
                    gcol = tbi * nts + s
                    for dh in range(NDH):
                        ps2 = pp2.tile([P, DH], f32)
                        for fc in range(NC2):
                            # lhsT: hT rows fc*256 + j*128 + p = F-tiles (2fc, 2fc+1)
                            nc.tensor.matmul(
                                ps2[:],
                                lhsT=h_sb[:, 2 * fc:2 * fc + 2, s * P:s * P + P],
                                rhs=w2_sb[fc][:, :, dh * DH:(dh + 1) * DH],
                                start=(fc == 0),
                                stop=(fc == NC2 - 1),
                                perf_mode=DR,
                            )
                        # psum holds 16*(h@w2); gt is pre-divided by 16
                        nc.scalar.mul(
                            o_sb[:, s, dh * DH:(dh + 1) * DH], ps2[:],
                            g_sb[:, gcol:gcol + 1],
                        )
                    nc.sync.dma_start(
                        out.ap()[tbi * tb + s * P: tbi * tb + (s + 1) * P, :],
                        o_sb[:, s, :],
                    )
    nc.compile()
    return nc


def _route_host(x, noise, w_route, b_route, w_noise, b_noise, top_k):
    """Replicates the oracle's router bit-exactly on CPU jax (op-for-op)."""
    import jax
    import jax.numpy as jnp

    cpu = jax.devices("cpu")[0]
    with jax.default_device(cpu):
        flat = jnp.asarray(np.asarray(x, np.float32)).reshape(-1, D)
        logits = (flat @ jnp.asarray(np.asarray(w_route, np.float32))
                  + jnp.asarray(np.asarray(b_route, np.float32))).T
        noise_logits = (flat @ jnp.asarray(np.asarray(w_noise, np.float32))
                        + jnp.asarray(np.asarray(b_noise, np.float32))).T
        noisy = logits + jnp.asarray(np.asarray(noise, np.float32)) * jax.nn.softplus(noise_logits)
        top_v, idx = jax.lax.top_k(noisy, top_k)
        gate = jax.nn.softmax(top_v, axis=-1)
        return np.asarray(idx), np.asarray(gate, np.float32)


def _gather_transpose(flat, idx):
    """[E,K] gather from flat [N,D] -> xgT [E, D, K] f32, via CPU jax."""
    import jax
    import jax.numpy as jnp

    cpu = jax.devices("cpu")[0]
    with jax.default_device(cpu):
        xg = jnp.take(jnp.asarray(flat), jnp.asarray(idx), axis=0)  # [E, K, D]
        xgT = jnp.transpose(xg, (0, 2, 1))
        return np.asarray(xgT)


def prepare(x, noise, w_route, b_route, w_noise, b_noise, w1, b1, w2, b2, top_k):
    """Host-side routing + sharding. Returns (build_key, in_maps, idx, flat)."""
    x = np.asarray(x, np.float32)
    w1 = np.asarray(w1, np.float32)
    b1 = np.asarray(b1, np.float32)
    w2 = np.asarray(w2, np.float32)
    b2 = np.asarray(b2, np.float32)
    assert int(top_k) == K

    idx, gate = _route_host(x, noise, w_route, b_route, w_noise, b_noise, int(top_k))
    flat = x.reshape(-1, D)
    xgT = _gather_transpose(flat, idx)

    has_b2 = bool(np.any(b2))
    has_b1 = bool(np.any(b1))
    in_maps = []
    if not has_b2:
        # fp8e4 DoubleRow path (w1,w2 pre-scaled by 16; gate divided by 16)
        key = "fp8_b1" if has_b1 else "fp8"
        f8 = ml_dtypes.float8_e4m3

        def to_f8(a):
            # clip to TRN fp8e4's +-240 range so outliers saturate, not inf
            return np.clip(a, -240.0, 240.0).astype(f8)

        for e in range(E):
            m = {
                "xgT": to_f8(xgT[e]),
                "w1": np.ascontiguousarray(
                    to_f8(w1[e] * 16).reshape(4, 2, P, F).transpose(0, 2, 1, 3)),
                "w2": np.ascontiguousarray(
                    to_f8(w2[e] * 16).reshape(16, 2, P, D).transpose(0, 2, 1, 3)),
                "gt": np.ascontiguousarray(
                    (gate[e] / 16.0).reshape(K // P, P).T.astype(np.float32)),
            }
            if has_b1:
                m["b1t"] = np.ascontiguousarray(
                    b1[e].reshape(NFT, P).T.astype(np.float32))
            in_maps.append(m)
    else:
        key = "bf16_b2"
        bf = ml_dtypes.bfloat16
        for e in range(E):
            m = {
                "xgT": xgT[e].astype(bf),
                "w1": w1[e].astype(bf),
                "w2": w2[e].astype(bf),
                "b1t": np.ascontiguousarray(b1[e].reshape(NFT, P).T.astype(np.float32)),
                "gt": np.ascontiguousarray(gate[e].reshape(K // P, P).T.astype(np.float32)),
            }
            m["b2r"] = b2[e].reshape(1, D).astype(bf)
            in_maps.append(m)

    return key, in_maps, idx, flat


def build_for(key):
    if key not in _STATE:
        if key == "fp8":
            _STATE[key] = build_nc_fp8(nb=16, tb=512, has_b1=False)
        elif key == "fp8_b1":
            _STATE[key] = build_nc_fp8(nb=16, tb=512, has_b1=True)
        else:
            _STATE[key] = build_nc(True)
    return _STATE[key]


def _run_device_subprocess(key, in_maps):
    """Disaster-recovery path: a device execution failure poisons the PJRT
    client for the rest of the process, but a fresh process's first
    execution recovers. Ship the per-core inputs to a new interpreter."""
    import os
    import subprocess
    import sys
    import tempfile

    tmp = tempfile.mkdtemp()
    inp, outp = os.path.join(tmp, "in.npz"), os.path.join(tmp, "out.npz")
    save = {}
    for e, m in enumerate(in_maps):
        for name, arr in m.items():
            dt = str(arr.dtype)
            save[f"{e}|{name}|{dt}"] = (
                arr if arr.dtype == np.float32 else arr.view(np.uint8))
    np.savez(inp, **save)
    kdir = os.path.dirname(os.path.abspath(__file__))
    runner = f"""
import numpy as np, ml_dtypes, sys
sys.path.insert(0, {kdir!r})
import kernel as kmod
from concourse.bass_utils import run_bass_kernel_spmd
z = np.load({inp!r})
in_maps = [dict() for _ in range(kmod.E)]
for kk in z.files:
    e, name, dt = kk.split('|')
    a = z[kk]
    if dt != 'float32':
        a = a.view(getattr(ml_dtypes, dt))
    in_maps[int(e)][name] = a
nc = kmod.build_for({key!r})
res = run_bass_kernel_spmd(nc, in_maps, core_ids=list(range(kmod.E)))
np.savez({outp!r}, **{{str(e): np.asarray(res.results[e]['out']).view(np.uint8)
                      for e in range(kmod.E)}})
"""
    subprocess.run([sys.executable, "-c", runner], check=True)
    z = np.load(outp)
    return [{"out": z[str(e)].view(ml_dtypes.bfloat16)} for e in range(E)]


def run_device(key, in_maps):
    from concourse.bass_utils import run_bass_kernel_spmd

    nc = build_for(key)
    for _ in range(2):
        try:
            return run_bass_kernel_spmd(nc, in_maps, core_ids=list(range(E))).results
        except Exception:
            pass
    return _run_device_subprocess(key, in_maps)


def kernel(x, noise, w_route, b_route, w_noise, b_noise, w1, b1, w2, b2, top_k):
    x = np.asarray(x, np.float32)
    B, S, _ = x.shape
    key, in_maps, idx, flat = prepare(
        x, noise, w_route, b_route, w_noise, b_noise, w1, b1, w2, b2, top_k)

    results = run_device(key, in_maps)

    final = flat.copy()
    for e in range(E):
        final[idx[e]] += np.asarray(results[e]["out"], dtype=np.float32)
    return final.reshape(B, S, D), idx


# revision 39
# speedup vs baseline: 1.0040x; 1.0013x over previous
"""Expert-choice token-sparse MoE for Trainium2 (8 NeuronCores, expert-parallel).

Contract: kernel(**inputs) takes the FULL unsharded inputs and returns the FULL
output, matching reference.reference(): a tuple (final [B,S,D] f32, idx [E,K] i32).

Sharding strategy (expert-parallel, per the hint):
  - The router + top-k runs on host CPU (bit-exact replication of the oracle's
    jax ops — idx is an integer output that must match exactly, which requires
    bit-identical fp32 routing scores; the heavy compute is NOT here).
  - Expert e's gathered tokens + weights are shipped to core e ("all-to-all
    dispatch" done host-side as part of sharding).
  - Each core runs the dense MLP for its 8192 selected tokens on the Tensor
    engine in fp8e4 DoubleRow (2 fp8 MACs/cell/cycle):
    hT = relu(w1^T @ xgT + b1); out = (hT^T @ w2) * gate  (~137 GFLOP/core).
    The fp8 quantization error is attenuated by the gate (~1e-4) relative to
    the fp32 residual, so final relative error stays ~2e-5. A bf16 variant
    handles the (spec-wise impossible) b2 != 0 case.
  - Host combines: scatter-add each expert's compact [K, D] output + residual.
Measured: ~911 us HW exec per core (92% MFU; fp8 roofline for the 137 GFLOP
is ~874 us + ~14 us DMA head + ~12 us drain tail).
"""

import numpy as np
import ml_dtypes

E = 8
D = 1024
F = 4096
N = 32768
K = 8192
P = 128
TB = 256              # tokens per device block
NDC = D // P          # 8 d-chunks
NFT = F // P          # 32 f-tiles
NB = K // TB          # 32 token blocks
NTS = TB // P         # 2 token subtiles per block
DH = 512              # out D half (PSUM bank)
NDH = D // DH         # 2

_STATE = {}


def build_nc(has_b2=False, nb=NB):
    import concourse.bacc as bacc
    import concourse.mybir as mybir
    import concourse.tile as tile

    bf16 = mybir.dt.bfloat16
    f32 = mybir.dt.float32
    Relu = mybir.ActivationFunctionType.Relu

    k = nb * TB
    nc = bacc.Bacc("TRN2", target_bir_lowering=False, debug=False)
    xgT = nc.dram_tensor("xgT", [D, k], bf16, kind="ExternalInput")
    w1 = nc.dram_tensor("w1", [D, F], bf16, kind="ExternalInput")
    w2 = nc.dram_tensor("w2", [F, D], bf16, kind="ExternalInput")
    b1t = nc.dram_tensor("b1t", [P, NFT], f32, kind="ExternalInput")
    gt = nc.dram_tensor("gt", [P, k // P], f32, kind="ExternalInput")
    if has_b2:
        b2r = nc.dram_tensor("b2r", [1, D], bf16, kind="ExternalInput")
    out = nc.dram_tensor("out", [k, D], bf16, kind="ExternalOutput")

    with tile.TileContext(nc) as tc:
        with (
            tc.tile_pool(name="wpool", bufs=1) as wpool,
            tc.tile_pool(name="xpool", bufs=3) as xpool,
            tc.tile_pool(name="hpool", bufs=2) as hpool,
            tc.tile_pool(name="opool", bufs=3) as opool,
            tc.tile_pool(name="pp1", bufs=4, space="PSUM") as pp1,
            tc.tile_pool(name="pp2", bufs=4, space="PSUM") as pp2,
        ):
            # resident weights
            w1_sb = []
            for c in range(NDC):
                t = wpool.tile([P, F], bf16, tag=f"w1_{c}")
                nc.sync.dma_start(t[:], w1.ap()[c * P:(c + 1) * P, :])
                w1_sb.append(t)
            w2_sb = []
            for c in range(NFT):
                t = wpool.tile([P, D], bf16, tag=f"w2_{c}")
                nc.sync.dma_start(t[:], w2.ap()[c * P:(c + 1) * P, :])
                w2_sb.append(t)
            b1_sb = wpool.tile([P, NFT], f32, tag="b1")
            nc.sync.dma_start(b1_sb[:], b1t.ap())
            g_sb = wpool.tile([P, k // P], f32, tag="g")
            nc.sync.dma_start(g_sb[:], gt.ap())
            if has_b2:
                b2_sb = wpool.tile([1, D], bf16, tag="b2")
                nc.sync.dma_start(b2_sb[:], b2r.ap())
                ones_sb = wpool.tile([1, P], bf16, tag="ones")
                nc.vector.memset(ones_sb[:], 1.0)

            xgT_r = xgT.ap().rearrange("(c p) t -> p c t", p=P)

            for tb in range(nb):
                xg_sb = xpool.tile([P, NDC, TB], bf16, tag="xg")
                nc.sync.dma_start(
                    xg_sb[:], xgT_r[:, :, tb * TB:(tb + 1) * TB]
                )

                h_sb = hpool.tile([P, NFT * TB], bf16, tag="h")
                for ft in range(NFT):
                    ps = pp1.tile([P, TB], f32)
                    for c in range(NDC):
                        nc.tensor.matmul(
                            ps[:],
                            lhsT=w1_sb[c][:, ft * P:(ft + 1) * P],
                            rhs=xg_sb[:, c, :],
                            start=(c == 0),
                            stop=(c == NDC - 1),
                        )
                    nc.scalar.activation(
                        h_sb[:, ft * TB:(ft + 1) * TB], ps[:], Relu,
                        bias=b1_sb[:, ft:ft + 1],
                    )

                o_sb = opool.tile([P, NTS, D], bf16, tag="o")
                for s in range(NTS):
                    gcol = tb * NTS + s
                    for dh in range(NDH):
                        ps2 = pp2.tile([P, DH], f32)
                        for fc in range(NFT):
                            nc.tensor.matmul(
                                ps2[:],
                                lhsT=h_sb[:, fc * TB + s * P: fc * TB + s * P + P],
                                rhs=w2_sb[fc][:, dh * DH:(dh + 1) * DH],
                                start=(fc == 0),
                                stop=(fc == NFT - 1 and not has_b2),
                            )
                        if has_b2:
                            nc.tensor.matmul(
                                ps2[:],
                                lhsT=ones_sb[:, :],
                                rhs=b2_sb[:, dh * DH:(dh + 1) * DH],
                                start=False,
                                stop=True,
                            )
                        nc.scalar.mul(
                            o_sb[:, s, dh * DH:(dh + 1) * DH], ps2[:],
                            g_sb[:, gcol:gcol + 1],
                        )
                    nc.sync.dma_start(
                        out.ap()[tb * TB + s * P: tb * TB + (s + 1) * P, :],
                        o_sb[:, s, :],
                    )
    nc.compile()
    return nc


def build_nc_fp8(nb=16, tb=512, has_b1=False):
    """fp8e4 DoubleRow variant: host pre-scales w1,w2 by 16 and gt by 1/16.
    Layouts: xgT8 [D,K] fp8 (k-blocks of 128 natural), w1dr [4,128,2,F] fp8,
    w2dr [16,128,2,D] fp8, b1t [128,32] f32 (only if has_b1), gt [128,K/128]
    f32 (pre-divided by 16). out [K,D] bf16. When b1 is all-zero (the spec
    case) the relu evictions use a float-const bias, removing the b1 DMA
    from the eviction dependency chain."""
    import concourse.bacc as bacc
    import concourse.mybir as mybir
    import concourse.tile as tile

    fp8 = mybir.dt.float8e4
    bf16 = mybir.dt.bfloat16
    f32 = mybir.dt.float32
    Relu = mybir.ActivationFunctionType.Relu
    DR = mybir.MatmulPerfMode.DoubleRow

    k = nb * tb
    nts = tb // P           # token subtiles per block
    NC1 = D // 256          # 4 contraction chunks (mm1)
    NC2 = F // 256          # 16 contraction chunks (mm2)

    nc = bacc.Bacc("TRN2", target_bir_lowering=False, debug=False)
    xgT = nc.dram_tensor("xgT", [D, k], fp8, kind="ExternalInput")
    w1 = nc.dram_tensor("w1", [NC1, P, 2, F], fp8, kind="ExternalInput")
    w2 = nc.dram_tensor("w2", [NC2, P, 2, D], fp8, kind="ExternalInput")
    if has_b1:
        b1t = nc.dram_tensor("b1t", [P, NFT], f32, kind="ExternalInput")
    gt = nc.dram_tensor("gt", [P, k // P], f32, kind="ExternalInput")
    out = nc.dram_tensor("out", [k, D], bf16, kind="ExternalOutput")

    with tile.TileContext(nc) as tc:
        with (
            tc.tile_pool(name="wpool", bufs=1) as wpool,
            tc.tile_pool(name="xpool", bufs=3) as xpool,
            tc.tile_pool(name="hpool", bufs=2) as hpool,
            tc.tile_pool(name="opool", bufs=3) as opool,
            tc.tile_pool(name="pp1", bufs=6, space="PSUM") as pp1,
            tc.tile_pool(name="pp2", bufs=2, space="PSUM") as pp2,
        ):
            # Warm the ACT engine's function tables (Relu for mm1 evictions,
            # Copy for the gate-scale evictions) during the DMA head — the
            # first use of an activation function pays a ~2us table load that
            # otherwise backs up PSUM and stalls the PE early on.
            warm = wpool.tile([1, 1], f32, tag="warm")
            warm_b = wpool.tile([1, 1], f32, tag="warm_b")
            nc.vector.memset(warm[:], 0.0)
            nc.vector.memset(warm_b[:], 0.0)
            nc.scalar.activation(warm[:], warm[:], Relu, bias=warm_b[:, :1])
            nc.scalar.mul(warm[:], warm[:], 1.0)

            # Explicit memset zero-bias for the relu evictions: a 0.0 float
            # bias would go through bass's const pool, whose backing DMA
            # lands at the END of the sync queue — evictions would then wait
            # behind the whole w2 load (~10us stall on the first block).
            zbias = wpool.tile([P, 1], f32, tag="zbias")
            nc.vector.memset(zbias[:], 0.0)

            # xgT rows: chunk c covers D-rows [c*256, (c+1)*256); slot j holds
            # rows c*256 + j*128 + p  ->  "(c j p) t"
            xgT_r = xgT.ap().rearrange("(c j p) t -> p c j t", j=2, p=P)

            def load_xg(tbi):
                t = xpool.tile([P, NC1, 2, tb], fp8, tag="xg")
                nc.sync.dma_start(t[:], xgT_r[:, :, :, tbi * tb:(tbi + 1) * tb])
                return t

            # DMA issue order matters for the pipeline head (descriptors drain
            # in order per queue at ~320GB/s): sync queue carries xg0 then w1
            # (first matmul needs exactly these) then xg1, then w2 (only
            # needed ~27us in). The tiny b1/g (mm1 evictions need them early)
            # go on the gpsimd queue so they don't add head latency.
            if has_b1:
                b1_sb = wpool.tile([P, NFT], f32, tag="b1")
                nc.gpsimd.dma_start(b1_sb[:], b1t.ap())
            g_sb = wpool.tile([P, k // P], f32, tag="g")
            nc.gpsimd.dma_start(g_sb[:], gt.ap())
            xg_pre = [load_xg(0)]
            w1_sb = []
            for c in range(NC1):
                t = wpool.tile([P, 2, F], fp8, tag=f"w1_{c}")
                nc.sync.dma_start(t[:], w1.ap()[c])
                w1_sb.append(t)
            if nb > 1:
                xg_pre.append(load_xg(1))
            w2_sb = []
            for c in range(NC2):
                t = wpool.tile([P, 2, D], fp8, tag=f"w2_{c}")
                nc.sync.dma_start(t[:], w2.ap()[c])
                w2_sb.append(t)

            for tbi in range(nb):
                xg_sb = xg_pre[tbi] if tbi < len(xg_pre) else load_xg(tbi)

                h_sb = hpool.tile([P, NFT, tb], fp8, tag="h")
                for ft in range(NFT):
                    ps = pp1.tile([P, tb], f32)
                    for c in range(NC1):
                        nc.tensor.matmul(
                            ps[:],
                            lhsT=w1_sb[c][:, :, ft * P:(ft + 1) * P],
                            rhs=xg_sb[:, c, :, :],
                            start=(c == 0),
                            stop=(c == NC1 - 1),
                            perf_mode=DR,
                        )
                    # psum holds 16*(x@w1); relu((psum/16) + b1)
                    nc.scalar.activation(
                        h_sb[:, ft, :], ps[:], Relu,
                        bias=(b1_sb[:, ft:ft + 1] if has_b1 else zbias[:, :1]),
                        scale=1.0 / 16.0,
                    )

                o_sb = opool.tile([P, nts, D], bf16, tag="o")
                for s in range(nts):
                    gcol = tbi * nts + s
                    for dh in range(NDH):
                        ps2 = pp2.tile([P, DH], f32)
                        for fc in range(NC2):
                            # lhsT: hT rows fc*256 + j*128 + p = F-tiles (2fc, 2fc+1)
                            nc.tensor.matmul(
                                ps2[:],
                                lhsT=h_sb[:, 2 * fc:2 * fc + 2, s * P:s * P + P],
                                rhs=w2_sb[fc][:, :, dh * DH:(dh + 1) * DH],
                                start=(fc == 0),
                                stop=(fc == NC2 - 1),
                                perf_mode=DR,
                            )
                        # psum holds 16*(h@w2); gt is pre-divided by 16
                        nc.scalar.mul(
                            o_sb[:, s, dh * DH:(dh + 1) * DH], ps2[:],
                            g_sb[:, gcol:gcol + 1],
                        )
                    nc.sync.dma_start(
                        out.ap()[tbi * tb + s * P: tbi * tb + (s + 1) * P, :],
                        o_sb[:, s, :],
                    )
    nc.compile()
    return nc


def _route_host(x, noise, w_route, b_route, w_noise, b_noise, top_k):
    """Replicates the oracle's router bit-exactly on CPU jax (op-for-op)."""
    import jax
    import jax.numpy as jnp

    cpu = jax.devices("cpu")[0]
    with jax.default_device(cpu):
        flat = jnp.asarray(np.asarray(x, np.float32)).reshape(-1, D)
        logits = (flat @ jnp.asarray(np.asarray(w_route, np.float32))
                  + jnp.asarray(np.asarray(b_route, np.float32))).T
        noise_logits = (flat @ jnp.asarray(np.asarray(w_noise, np.float32))
                        + jnp.asarray(np.asarray(b_noise, np.float32))).T
        noisy = logits + jnp.asarray(np.asarray(noise, np.float32)) * jax.nn.softplus(noise_logits)
        top_v, idx = jax.lax.top_k(noisy, top_k)
        gate = jax.nn.softmax(top_v, axis=-1)
        return np.asarray(idx), np.asarray(gate, np.float32)


def _gather_transpose(flat, idx):
    """[E,K] gather from flat [N,D] -> xgT [E, D, K] f32, via CPU jax."""
    import jax
    import jax.numpy as jnp

    cpu = jax.devices("cpu")[0]
    with jax.default_device(cpu):
        xg = jnp.take(jnp.asarray(flat), jnp.asarray(idx), axis=0)  # [E, K, D]
        xgT = jnp.transpose(xg, (0, 2, 1))
        return np.asarray(xgT)


def prepare(x, noise, w_route, b_route, w_noise, b_noise, w1, b1, w2, b2, top_k):
    """Host-side routing + sharding. Returns (build_key, in_maps, idx, flat)."""
    x = np.asarray(x, np.float32)
    w1 = np.asarray(w1, np.float32)
    b1 = np.asarray(b1, np.float32)
    w2 = np.asarray(w2, np.float32)
    b2 = np.asarray(b2, np.float32)
    assert int(top_k) == K

    idx, gate = _route_host(x, noise, w_route, b_route, w_noise, b_noise, int(top_k))
    flat = x.reshape(-1, D)
    xgT = _gather_transpose(flat, idx)

    has_b2 = bool(np.any(b2))
    has_b1 = bool(np.any(b1))
    in_maps = []
    if not has_b2:
        # fp8e4 DoubleRow path (w1,w2 pre-scaled by 16; gate divided by 16)
        key = "fp8_b1" if has_b1 else "fp8"
        f8 = ml_dtypes.float8_e4m3

        def to_f8(a):
            # clip to TRN fp8e4's +-240 range so outliers saturate, not inf
            return np.clip(a, -240.0, 240.0).astype(f8)

        for e in range(E):
            m = {
                "xgT": to_f8(xgT[e]),
                "w1": np.ascontiguousarray(
                    to_f8(w1[e] * 16).reshape(4, 2, P, F).transpose(0, 2, 1, 3)),
                "w2": np.ascontiguousarray(
                    to_f8(w2[e] * 16).reshape(16, 2, P, D).transpose(0, 2, 1, 3)),
                "gt": np.ascontiguousarray(
                    (gate[e] / 16.0).reshape(K // P, P).T.astype(np.float32)),
            }
            if has_b1:
                m["b1t"] = np.ascontiguousarray(
                    b1[e].reshape(NFT, P).T.astype(np.float32))
            in_maps.append(m)
    else:
        key = "bf16_b2"
        bf = ml_dtypes.bfloat16
        for e in range(E):
            m = {
                "xgT": xgT[e].astype(bf),
                "w1": w1[e].astype(bf),
                "w2": w2[e].astype(bf),
                "b1t": np.ascontiguousarray(b1[e].reshape(NFT, P).T.astype(np.float32)),
                "gt": np.ascontiguousarray(gate[e].reshape(K // P, P).T.astype(np.float32)),
            }
            m["b2r"] = b2[e].reshape(1, D).astype(bf)
            in_maps.append(m)

    return key, in_maps, idx, flat


def build_for(key):
    if key not in _STATE:
        if key == "fp8":
            _STATE[key] = build_nc_fp8(nb=16, tb=512, has_b1=False)
        elif key == "fp8_b1":
            _STATE[key] = build_nc_fp8(nb=16, tb=512, has_b1=True)
        else:
            _STATE[key] = build_nc(True)
    return _STATE[key]


def _run_device_subprocess(key, in_maps):
    """Disaster-recovery path: a device execution failure poisons the PJRT
    client for the rest of the process, but a fresh process's first
    execution recovers. Ship the per-core inputs to a new interpreter."""
    import os
    import subprocess
    import sys
    import tempfile

    tmp = tempfile.mkdtemp()
    inp, outp = os.path.join(tmp, "in.npz"), os.path.join(tmp, "out.npz")
    save = {}
    for e, m in enumerate(in_maps):
        for name, arr in m.items():
            dt = str(arr.dtype)
            save[f"{e}|{name}|{dt}"] = (
                arr if arr.dtype == np.float32 else arr.view(np.uint8))
    np.savez(inp, **save)
    kdir = os.path.dirname(os.path.abspath(__file__))
    runner = f"""
import numpy as np, ml_dtypes, sys
sys.path.insert(0, {kdir!r})
import kernel as kmod
from concourse.bass_utils import run_bass_kernel_spmd
z = np.load({inp!r})
in_maps = [dict() for _ in range(kmod.E)]
for kk in z.files:
    e, name, dt = kk.split('|')
    a = z[kk]
    if dt != 'float32':
        a = a.view(getattr(ml_dtypes, dt))
    in_maps[int(e)][name] = a
nc = kmod.build_for({key!r})
res = run_bass_kernel_spmd(nc, in_maps, core_ids=list(range(kmod.E)))
np.savez({outp!r}, **{{str(e): np.asarray(res.results[e]['out']).view(np.uint8)
                      for e in range(kmod.E)}})
"""
    subprocess.run([sys.executable, "-c", runner], check=True)
    z = np.load(outp)
    return [{"out": z[str(e)].view(ml_dtypes.bfloat16)} for e in range(E)]


def run_device(key, in_maps):
    from concourse.bass_utils import run_bass_kernel_spmd

    nc = build_for(key)
    for _ in range(2):
        try:
            return run_bass_kernel_spmd(nc, in_maps, core_ids=list(range(E))).results
        except Exception:
            pass
    return _run_device_subprocess(key, in_maps)


def kernel(x, noise, w_route, b_route, w_noise, b_noise, w1, b1, w2, b2, top_k):
    x = np.asarray(x, np.float32)
    B, S, _ = x.shape
    key, in_maps, idx, flat = prepare(
        x, noise, w_route, b_route, w_noise, b_noise, w1, b1, w2, b2, top_k)

    results = run_device(key, in_maps)

    final = flat.copy()
    for e in range(E):
        final[idx[e]] += np.asarray(results[e]["out"], dtype=np.float32)
    return final.reshape(B, S, D), idx


# revision 40
# speedup vs baseline: 1.0054x; 1.0014x over previous
"""Expert-choice token-sparse MoE for Trainium2 (8 NeuronCores, expert-parallel).

Contract: kernel(**inputs) takes the FULL unsharded inputs and returns the FULL
output, matching reference.reference(): a tuple (final [B,S,D] f32, idx [E,K] i32).

Sharding strategy (expert-parallel, per the hint):
  - The router + top-k runs on host CPU (bit-exact replication of the oracle's
    jax ops — idx is an integer output that must match exactly, which requires
    bit-identical fp32 routing scores; the heavy compute is NOT here).
  - Expert e's gathered tokens + weights are shipped to core e ("all-to-all
    dispatch" done host-side as part of sharding).
  - Each core runs the dense MLP for its 8192 selected tokens on the Tensor
    engine in fp8e4 DoubleRow (2 fp8 MACs/cell/cycle):
    hT = relu(w1^T @ xgT + b1); out = (hT^T @ w2) * gate  (~137 GFLOP/core).
    The fp8 quantization error is attenuated by the gate (~1e-4) relative to
    the fp32 residual, so final relative error stays ~2e-5. A bf16 variant
    handles the (spec-wise impossible) b2 != 0 case.
  - Host combines: scatter-add each expert's compact [K, D] output + residual.
Measured: ~911 us HW exec per core (92% MFU; fp8 roofline for the 137 GFLOP
is ~874 us + ~14 us DMA head + ~12 us drain tail).
"""

import numpy as np
import ml_dtypes

E = 8
D = 1024
F = 4096
N = 32768
K = 8192
P = 128
TB = 256              # tokens per device block
NDC = D // P          # 8 d-chunks
NFT = F // P          # 32 f-tiles
NB = K // TB          # 32 token blocks
NTS = TB // P         # 2 token subtiles per block
DH = 512              # out D half (PSUM bank)
NDH = D // DH         # 2

_STATE = {}


def build_nc(has_b2=False, nb=NB):
    import concourse.bacc as bacc
    import concourse.mybir as mybir
    import concourse.tile as tile

    bf16 = mybir.dt.bfloat16
    f32 = mybir.dt.float32
    Relu = mybir.ActivationFunctionType.Relu

    k = nb * TB
    nc = bacc.Bacc("TRN2", target_bir_lowering=False, debug=False)
    xgT = nc.dram_tensor("xgT", [D, k], bf16, kind="ExternalInput")
    w1 = nc.dram_tensor("w1", [D, F], bf16, kind="ExternalInput")
    w2 = nc.dram_tensor("w2", [F, D], bf16, kind="ExternalInput")
    b1t = nc.dram_tensor("b1t", [P, NFT], f32, kind="ExternalInput")
    gt = nc.dram_tensor("gt", [P, k // P], f32, kind="ExternalInput")
    if has_b2:
        b2r = nc.dram_tensor("b2r", [1, D], bf16, kind="ExternalInput")
    out = nc.dram_tensor("out", [k, D], bf16, kind="ExternalOutput")

    with tile.TileContext(nc) as tc:
        with (
            tc.tile_pool(name="wpool", bufs=1) as wpool,
            tc.tile_pool(name="xpool", bufs=3) as xpool,
            tc.tile_pool(name="hpool", bufs=2) as hpool,
            tc.tile_pool(name="opool", bufs=3) as opool,
            tc.tile_pool(name="pp1", bufs=4, space="PSUM") as pp1,
            tc.tile_pool(name="pp2", bufs=4, space="PSUM") as pp2,
        ):
            # resident weights
            w1_sb = []
            for c in range(NDC):
                t = wpool.tile([P, F], bf16, tag=f"w1_{c}")
                nc.sync.dma_start(t[:], w1.ap()[c * P:(c + 1) * P, :])
                w1_sb.append(t)
            w2_sb = []
            for c in range(NFT):
                t = wpool.tile([P, D], bf16, tag=f"w2_{c}")
                nc.sync.dma_start(t[:], w2.ap()[c * P:(c + 1) * P, :])
                w2_sb.append(t)
            b1_sb = wpool.tile([P, NFT], f32, tag="b1")
            nc.sync.dma_start(b1_sb[:], b1t.ap())
            g_sb = wpool.tile([P, k // P], f32, tag="g")
            nc.sync.dma_start(g_sb[:], gt.ap())
            if has_b2:
                b2_sb = wpool.tile([1, D], bf16, tag="b2")
                nc.sync.dma_start(b2_sb[:], b2r.ap())
                ones_sb = wpool.tile([1, P], bf16, tag="ones")
                nc.vector.memset(ones_sb[:], 1.0)

            xgT_r = xgT.ap().rearrange("(c p) t -> p c t", p=P)

            for tb in range(nb):
                xg_sb = xpool.tile([P, NDC, TB], bf16, tag="xg")
                nc.sync.dma_start(
                    xg_sb[:], xgT_r[:, :, tb * TB:(tb + 1) * TB]
                )

                h_sb = hpool.tile([P, NFT * TB], bf16, tag="h")
                for ft in range(NFT):
                    ps = pp1.tile([P, TB], f32)
                    for c in range(NDC):
                        nc.tensor.matmul(
                            ps[:],
                            lhsT=w1_sb[c][:, ft * P:(ft + 1) * P],
                            rhs=xg_sb[:, c, :],
                            start=(c == 0),
                            stop=(c == NDC - 1),
                        )
                    nc.scalar.activation(
                        h_sb[:, ft * TB:(ft + 1) * TB], ps[:], Relu,
                        bias=b1_sb[:, ft:ft + 1],
                    )

                o_sb = opool.tile([P, NTS, D], bf16, tag="o")
                for s in range(NTS):
                    gcol = tb * NTS + s
                    for dh in range(NDH):
                        ps2 = pp2.tile([P, DH], f32)
                        for fc in range(NFT):
                            nc.tensor.matmul(
                                ps2[:],
                                lhsT=h_sb[:, fc * TB + s * P: fc * TB + s * P + P],
                                rhs=w2_sb[fc][:, dh * DH:(dh + 1) * DH],
                                start=(fc == 0),
                                stop=(fc == NFT - 1 and not has_b2),
                            )
                        if has_b2:
                            nc.tensor.matmul(
                                ps2[:],
                                lhsT=ones_sb[:, :],
                                rhs=b2_sb[:, dh * DH:(dh + 1) * DH],
                                start=False,
                                stop=True,
                            )
                        nc.scalar.mul(
                            o_sb[:, s, dh * DH:(dh + 1) * DH], ps2[:],
                            g_sb[:, gcol:gcol + 1],
                        )
                    nc.sync.dma_start(
                        out.ap()[tb * TB + s * P: tb * TB + (s + 1) * P, :],
                        o_sb[:, s, :],
                    )
    nc.compile()
    return nc


def build_nc_fp8(nb=16, tb=512, has_b1=False):
    """fp8e4 DoubleRow variant: host pre-scales w1,w2 by 16 and gt by 1/16.
    Layouts: xgT8 [D,K] fp8 (k-blocks of 128 natural), w1dr [4,128,2,F] fp8,
    w2dr [16,128,2,D] fp8, b1t [128,32] f32 (only if has_b1), gt [128,K/128]
    f32 (pre-divided by 16). out [K,D] bf16. When b1 is all-zero (the spec
    case) the relu evictions use a float-const bias, removing the b1 DMA
    from the eviction dependency chain."""
    import concourse.bacc as bacc
    import concourse.mybir as mybir
    import concourse.tile as tile

    fp8 = mybir.dt.float8e4
    bf16 = mybir.dt.bfloat16
    f32 = mybir.dt.float32
    Relu = mybir.ActivationFunctionType.Relu
    DR = mybir.MatmulPerfMode.DoubleRow

    k = nb * tb
    nts = tb // P           # token subtiles per block
    NC1 = D // 256          # 4 contraction chunks (mm1)
    NC2 = F // 256          # 16 contraction chunks (mm2)

    nc = bacc.Bacc("TRN2", target_bir_lowering=False, debug=False)
    xgT = nc.dram_tensor("xgT", [D, k], fp8, kind="ExternalInput")
    w1 = nc.dram_tensor("w1", [NC1, P, 2, F], fp8, kind="ExternalInput")
    w2 = nc.dram_tensor("w2", [NC2, P, 2, D], fp8, kind="ExternalInput")
    if has_b1:
        b1t = nc.dram_tensor("b1t", [P, NFT], f32, kind="ExternalInput")
    gt = nc.dram_tensor("gt", [P, k // P], f32, kind="ExternalInput")
    out = nc.dram_tensor("out", [k, D], bf16, kind="ExternalOutput")

    with tile.TileContext(nc) as tc:
        with (
            tc.tile_pool(name="wpool", bufs=1) as wpool,
            tc.tile_pool(name="xpool", bufs=3) as xpool,
            tc.tile_pool(name="hpool", bufs=2) as hpool,
            tc.tile_pool(name="opool", bufs=3) as opool,
            tc.tile_pool(name="pp1", bufs=6, space="PSUM") as pp1,
            tc.tile_pool(name="pp2", bufs=2, space="PSUM") as pp2,
        ):
            # Warm the ACT engine's function tables (Relu for mm1 evictions,
            # Copy for the gate-scale evictions) during the DMA head — the
            # first use of an activation function pays a ~2us table load that
            # otherwise backs up PSUM and stalls the PE early on.
            warm = wpool.tile([1, 1], f32, tag="warm")
            warm_b = wpool.tile([1, 1], f32, tag="warm_b")
            nc.vector.memset(warm[:], 0.0)
            nc.vector.memset(warm_b[:], 0.0)
            nc.scalar.activation(warm[:], warm[:], Relu, bias=warm_b[:, :1])
            nc.scalar.mul(warm[:], warm[:], 1.0)

            # Explicit memset zero-bias for the relu evictions: a 0.0 float
            # bias would go through bass's const pool, whose backing DMA
            # lands at the END of the sync queue — evictions would then wait
            # behind the whole w2 load (~10us stall on the first block).
            zbias = wpool.tile([P, 1], f32, tag="zbias")
            nc.vector.memset(zbias[:], 0.0)
            # scratch operands for PE warmup matmuls
            wt = wpool.tile([P, 2, 512], fp8, tag="wt")
            nc.vector.memset(wt[:], 0.0)

            # xgT rows: chunk c covers D-rows [c*256, (c+1)*256); slot j holds
            # rows c*256 + j*128 + p  ->  "(c j p) t"
            xgT_r = xgT.ap().rearrange("(c j p) t -> p c j t", j=2, p=P)

            def load_xg(tbi):
                t = xpool.tile([P, NC1, 2, tb], fp8, tag="xg")
                nc.sync.dma_start(t[:], xgT_r[:, :, :, tbi * tb:(tbi + 1) * tb])
                return t

            # DMA issue order matters for the pipeline head (descriptors drain
            # in order per queue at ~320GB/s): sync queue carries xg0 then w1
            # (first matmul needs exactly these) then xg1, then w2 (only
            # needed ~27us in). The tiny b1/g (mm1 evictions need them early)
            # go on the gpsimd queue so they don't add head latency.
            if has_b1:
                b1_sb = wpool.tile([P, NFT], f32, tag="b1")
                nc.gpsimd.dma_start(b1_sb[:], b1t.ap())
            g_sb = wpool.tile([P, k // P], f32, tag="g")
            nc.gpsimd.dma_start(g_sb[:], gt.ap())
            xg_pre = [load_xg(0)]
            w1_sb = []
            for c in range(NC1):
                t = wpool.tile([P, 2, F], fp8, tag=f"w1_{c}")
                nc.sync.dma_start(t[:], w1.ap()[c])
                w1_sb.append(t)
            if nb > 1:
                xg_pre.append(load_xg(1))
            w2_sb = []
            for c in range(NC2):
                t = wpool.tile([P, 2, D], fp8, tag=f"w2_{c}")
                nc.sync.dma_start(t[:], w2.ap()[c])
                w2_sb.append(t)

            # PE warmup: ~16 throwaway matmuls on memset scratch bridge the
            # HAM activity window (cold 1.2GHz -> warm 2.4GHz flip needs
            # ~3.4us of sustained PE busy) while the w1 DMA streams in, so
            # the real stream starts at full clock.
            for i in range(16):
                psw = pp1.tile([P, tb], f32, tag="ps", name=f"psw_{i}")
                nc.tensor.matmul(
                    psw[:, :256], lhsT=wt[:, :, 0:128], rhs=wt[:, :, 0:256],
                    start=True, stop=True, perf_mode=DR,
                )

            for tbi in range(nb):
                xg_sb = xg_pre[tbi] if tbi < len(xg_pre) else load_xg(tbi)

                h_sb = hpool.tile([P, NFT, tb], fp8, tag="h")

                def evict(ft, ps):
                    # psum holds 16*(x@w1); relu((psum/16) + b1)
                    nc.scalar.activation(
                        h_sb[:, ft, :], ps[:], Relu,
                        bias=(b1_sb[:, ft:ft + 1] if has_b1 else zbias[:, :1]),
                        scale=1.0 / 16.0,
                    )

                if tbi == 0:
                    # block 0 runs c-outer in ft-groups of 6 (PSUM slots), so
                    # each w1 chunk's matmuls issue as soon as that chunk's
                    # DMA lands instead of waiting for the whole w1
                    GSZ = 6
                    for g0 in range(0, NFT, GSZ):
                        fts = list(range(g0, min(g0 + GSZ, NFT)))
                        pss = {
                            ft: pp1.tile([P, tb], f32, tag="ps", name=f"ps0_{ft}")
                            for ft in fts
                        }
                        for c in range(NC1):
                            for ft in fts:
                                nc.tensor.matmul(
                                    pss[ft][:],
                                    lhsT=w1_sb[c][:, :, ft * P:(ft + 1) * P],
                                    rhs=xg_sb[:, c, :, :],
                                    start=(c == 0),
                                    stop=(c == NC1 - 1),
                                    perf_mode=DR,
                                )
                        for ft in fts:
                            evict(ft, pss[ft])
                else:
                    for ft in range(NFT):
                        ps = pp1.tile([P, tb], f32)
                        for c in range(NC1):
                            nc.tensor.matmul(
                                ps[:],
                                lhsT=w1_sb[c][:, :, ft * P:(ft + 1) * P],
                                rhs=xg_sb[:, c, :, :],
                                start=(c == 0),
                                stop=(c == NC1 - 1),
                                perf_mode=DR,
                            )
                        evict(ft, ps)

                o_sb = opool.tile([P, nts, D], bf16, tag="o")
                for s in range(nts):
                    gcol = tbi * nts + s
                    for dh in range(NDH):
                        ps2 = pp2.tile([P, DH], f32)
                        for fc in range(NC2):
                            # lhsT: hT rows fc*256 + j*128 + p = F-tiles (2fc, 2fc+1)
                            nc.tensor.matmul(
                                ps2[:],
                                lhsT=h_sb[:, 2 * fc:2 * fc + 2, s * P:s * P + P],
                                rhs=w2_sb[fc][:, :, dh * DH:(dh + 1) * DH],
                                start=(fc == 0),
                                stop=(fc == NC2 - 1),
                                perf_mode=DR,
                            )
                        # psum holds 16*(h@w2); gt is pre-divided by 16
                        nc.scalar.mul(
                            o_sb[:, s, dh * DH:(dh + 1) * DH], ps2[:],
                            g_sb[:, gcol:gcol + 1],
                        )
                    nc.sync.dma_start(
                        out.ap()[tbi * tb + s * P: tbi * tb + (s + 1) * P, :],
                        o_sb[:, s, :],
                    )
    nc.compile()
    return nc


def _route_host(x, noise, w_route, b_route, w_noise, b_noise, top_k):
    """Replicates the oracle's router bit-exactly on CPU jax (op-for-op)."""
    import jax
    import jax.numpy as jnp

    cpu = jax.devices("cpu")[0]
    with jax.default_device(cpu):
        flat = jnp.asarray(np.asarray(x, np.float32)).reshape(-1, D)
        logits = (flat @ jnp.asarray(np.asarray(w_route, np.float32))
                  + jnp.asarray(np.asarray(b_route, np.float32))).T
        noise_logits = (flat @ jnp.asarray(np.asarray(w_noise, np.float32))
                        + jnp.asarray(np.asarray(b_noise, np.float32))).T
        noisy = logits + jnp.asarray(np.asarray(noise, np.float32)) * jax.nn.softplus(noise_logits)
        top_v, idx = jax.lax.top_k(noisy, top_k)
        gate = jax.nn.softmax(top_v, axis=-1)
        return np.asarray(idx), np.asarray(gate, np.float32)


def _gather_transpose(flat, idx):
    """[E,K] gather from flat [N,D] -> xgT [E, D, K] f32, via CPU jax."""
    import jax
    import jax.numpy as jnp

    cpu = jax.devices("cpu")[0]
    with jax.default_device(cpu):
        xg = jnp.take(jnp.asarray(flat), jnp.asarray(idx), axis=0)  # [E, K, D]
        xgT = jnp.transpose(xg, (0, 2, 1))
        return np.asarray(xgT)


def prepare(x, noise, w_route, b_route, w_noise, b_noise, w1, b1, w2, b2, top_k):
    """Host-side routing + sharding. Returns (build_key, in_maps, idx, flat)."""
    x = np.asarray(x, np.float32)
    w1 = np.asarray(w1, np.float32)
    b1 = np.asarray(b1, np.float32)
    w2 = np.asarray(w2, np.float32)
    b2 = np.asarray(b2, np.float32)
    assert int(top_k) == K

    idx, gate = _route_host(x, noise, w_route, b_route, w_noise, b_noise, int(top_k))
    flat = x.reshape(-1, D)
    xgT = _gather_transpose(flat, idx)

    has_b2 = bool(np.any(b2))
    has_b1 = bool(np.any(b1))
    in_maps = []
    if not has_b2:
        # fp8e4 DoubleRow path (w1,w2 pre-scaled by 16; gate divided by 16)
        key = "fp8_b1" if has_b1 else "fp8"
        f8 = ml_dtypes.float8_e4m3

        def to_f8(a):
            # clip to TRN fp8e4's +-240 range so outliers saturate, not inf
            return np.clip(a, -240.0, 240.0).astype(f8)

        for e in range(E):
            m = {
                "xgT": to_f8(xgT[e]),
                "w1": np.ascontiguousarray(
                    to_f8(w1[e] * 16).reshape(4, 2, P, F).transpose(0, 2, 1, 3)),
                "w2": np.ascontiguousarray(
                    to_f8(w2[e] * 16).reshape(16, 2, P, D).transpose(0, 2, 1, 3)),
                "gt": np.ascontiguousarray(
                    (gate[e] / 16.0).reshape(K // P, P).T.astype(np.float32)),
            }
            if has_b1:
                m["b1t"] = np.ascontiguousarray(
                    b1[e].reshape(NFT, P).T.astype(np.float32))
            in_maps.append(m)
    else:
        key = "bf16_b2"
        bf = ml_dtypes.bfloat16
        for e in range(E):
            m = {
                "xgT": xgT[e].astype(bf),
                "w1": w1[e].astype(bf),
                "w2": w2[e].astype(bf),
                "b1t": np.ascontiguousarray(b1[e].reshape(NFT, P).T.astype(np.float32)),
                "gt": np.ascontiguousarray(gate[e].reshape(K // P, P).T.astype(np.float32)),
            }
            m["b2r"] = b2[e].reshape(1, D).astype(bf)
            in_maps.append(m)

    return key, in_maps, idx, flat


def build_for(key):
    if key not in _STATE:
        if key == "fp8":
            _STATE[key] = build_nc_fp8(nb=16, tb=512, has_b1=False)
        elif key == "fp8_b1":
            _STATE[key] = build_nc_fp8(nb=16, tb=512, has_b1=True)
        else:
            _STATE[key] = build_nc(True)
    return _STATE[key]


def _run_device_subprocess(key, in_maps):
    """Disaster-recovery path: a device execution failure poisons the PJRT
    client for the rest of the process, but a fresh process's first
    execution recovers. Ship the per-core inputs to a new interpreter."""
    import os
    import subprocess
    import sys
    import tempfile

    tmp = tempfile.mkdtemp()
    inp, outp = os.path.join(tmp, "in.npz"), os.path.join(tmp, "out.npz")
    save = {}
    for e, m in enumerate(in_maps):
        for name, arr in m.items():
            dt = str(arr.dtype)
            save[f"{e}|{name}|{dt}"] = (
                arr if arr.dtype == np.float32 else arr.view(np.uint8))
    np.savez(inp, **save)
    kdir = os.path.dirname(os.path.abspath(__file__))
    runner = f"""
import numpy as np, ml_dtypes, sys
sys.path.insert(0, {kdir!r})
import kernel as kmod
from concourse.bass_utils import run_bass_kernel_spmd
z = np.load({inp!r})
in_maps = [dict() for _ in range(kmod.E)]
for kk in z.files:
    e, name, dt = kk.split('|')
    a = z[kk]
    if dt != 'float32':
        a = a.view(getattr(ml_dtypes, dt))
    in_maps[int(e)][name] = a
nc = kmod.build_for({key!r})
res = run_bass_kernel_spmd(nc, in_maps, core_ids=list(range(kmod.E)))
np.savez({outp!r}, **{{str(e): np.asarray(res.results[e]['out']).view(np.uint8)
                      for e in range(kmod.E)}})
"""
    subprocess.run([sys.executable, "-c", runner], check=True)
    z = np.load(outp)
    return [{"out": z[str(e)].view(ml_dtypes.bfloat16)} for e in range(E)]


def run_device(key, in_maps):
    from concourse.bass_utils import run_bass_kernel_spmd

    nc = build_for(key)
    for _ in range(2):
        try:
            return run_bass_kernel_spmd(nc, in_maps, core_ids=list(range(E))).results
        except Exception:
            pass
    return _run_device_subprocess(key, in_maps)


def kernel(x, noise, w_route, b_route, w_noise, b_noise, w1, b1, w2, b2, top_k):
    x = np.asarray(x, np.float32)
    B, S, _ = x.shape
    key, in_maps, idx, flat = prepare(
        x, noise, w_route, b_route, w_noise, b_noise, w1, b1, w2, b2, top_k)

    results = run_device(key, in_maps)

    final = flat.copy()
    for e in range(E):
        final[idx[e]] += np.asarray(results[e]["out"], dtype=np.float32)
    return final.reshape(B, S, D), idx


# revision 41
# speedup vs baseline: 1.0057x; 1.0003x over previous
"""Expert-choice token-sparse MoE for Trainium2 (8 NeuronCores, expert-parallel).

Contract: kernel(**inputs) takes the FULL unsharded inputs and returns the FULL
output, matching reference.reference(): a tuple (final [B,S,D] f32, idx [E,K] i32).

Sharding strategy (expert-parallel, per the hint):
  - The router + top-k runs on host CPU (bit-exact replication of the oracle's
    jax ops — idx is an integer output that must match exactly, which requires
    bit-identical fp32 routing scores; the heavy compute is NOT here).
  - Expert e's gathered tokens + weights are shipped to core e ("all-to-all
    dispatch" done host-side as part of sharding).
  - Each core runs the dense MLP for its 8192 selected tokens on the Tensor
    engine in fp8e4 DoubleRow (2 fp8 MACs/cell/cycle):
    hT = relu(w1^T @ xgT + b1); out = (hT^T @ w2) * gate  (~137 GFLOP/core).
    The fp8 quantization error is attenuated by the gate (~1e-4) relative to
    the fp32 residual, so final relative error stays ~2e-5. A bf16 variant
    handles the (spec-wise impossible) b2 != 0 case.
  - Host combines: scatter-add each expert's compact [K, D] output + residual.
Measured: ~911 us HW exec per core (92% MFU; fp8 roofline for the 137 GFLOP
is ~874 us + ~14 us DMA head + ~12 us drain tail).
"""

import numpy as np
import ml_dtypes

E = 8
D = 1024
F = 4096
N = 32768
K = 8192
P = 128
TB = 256              # tokens per device block
NDC = D // P          # 8 d-chunks
NFT = F // P          # 32 f-tiles
NB = K // TB          # 32 token blocks
NTS = TB // P         # 2 token subtiles per block
DH = 512              # out D half (PSUM bank)
NDH = D // DH         # 2

_STATE = {}


def build_nc(has_b2=False, nb=NB):
    import concourse.bacc as bacc
    import concourse.mybir as mybir
    import concourse.tile as tile

    bf16 = mybir.dt.bfloat16
    f32 = mybir.dt.float32
    Relu = mybir.ActivationFunctionType.Relu

    k = nb * TB
    nc = bacc.Bacc("TRN2", target_bir_lowering=False, debug=False)
    xgT = nc.dram_tensor("xgT", [D, k], bf16, kind="ExternalInput")
    w1 = nc.dram_tensor("w1", [D, F], bf16, kind="ExternalInput")
    w2 = nc.dram_tensor("w2", [F, D], bf16, kind="ExternalInput")
    b1t = nc.dram_tensor("b1t", [P, NFT], f32, kind="ExternalInput")
    gt = nc.dram_tensor("gt", [P, k // P], f32, kind="ExternalInput")
    if has_b2:
        b2r = nc.dram_tensor("b2r", [1, D], bf16, kind="ExternalInput")
    out = nc.dram_tensor("out", [k, D], bf16, kind="ExternalOutput")

    with tile.TileContext(nc) as tc:
        with (
            tc.tile_pool(name="wpool", bufs=1) as wpool,
            tc.tile_pool(name="xpool", bufs=3) as xpool,
            tc.tile_pool(name="hpool", bufs=2) as hpool,
            tc.tile_pool(name="opool", bufs=3) as opool,
            tc.tile_pool(name="pp1", bufs=4, space="PSUM") as pp1,
            tc.tile_pool(name="pp2", bufs=4, space="PSUM") as pp2,
        ):
            # resident weights
            w1_sb = []
            for c in range(NDC):
                t = wpool.tile([P, F], bf16, tag=f"w1_{c}")
                nc.sync.dma_start(t[:], w1.ap()[c * P:(c + 1) * P, :])
                w1_sb.append(t)
            w2_sb = []
            for c in range(NFT):
                t = wpool.tile([P, D], bf16, tag=f"w2_{c}")
                nc.sync.dma_start(t[:], w2.ap()[c * P:(c + 1) * P, :])
                w2_sb.append(t)
            b1_sb = wpool.tile([P, NFT], f32, tag="b1")
            nc.sync.dma_start(b1_sb[:], b1t.ap())
            g_sb = wpool.tile([P, k // P], f32, tag="g")
            nc.sync.dma_start(g_sb[:], gt.ap())
            if has_b2:
                b2_sb = wpool.tile([1, D], bf16, tag="b2")
                nc.sync.dma_start(b2_sb[:], b2r.ap())
                ones_sb = wpool.tile([1, P], bf16, tag="ones")
                nc.vector.memset(ones_sb[:], 1.0)

            xgT_r = xgT.ap().rearrange("(c p) t -> p c t", p=P)

            for tb in range(nb):
                xg_sb = xpool.tile([P, NDC, TB], bf16, tag="xg")
                nc.sync.dma_start(
                    xg_sb[:], xgT_r[:, :, tb * TB:(tb + 1) * TB]
                )

                h_sb = hpool.tile([P, NFT * TB], bf16, tag="h")
                for ft in range(NFT):
                    ps = pp1.tile([P, TB], f32)
                    for c in range(NDC):
                        nc.tensor.matmul(
                            ps[:],
                            lhsT=w1_sb[c][:, ft * P:(ft + 1) * P],
                            rhs=xg_sb[:, c, :],
                            start=(c == 0),
                            stop=(c == NDC - 1),
                        )
                    nc.scalar.activation(
                        h_sb[:, ft * TB:(ft + 1) * TB], ps[:], Relu,
                        bias=b1_sb[:, ft:ft + 1],
                    )

                o_sb = opool.tile([P, NTS, D], bf16, tag="o")
                for s in range(NTS):
                    gcol = tb * NTS + s
                    for dh in range(NDH):
                        ps2 = pp2.tile([P, DH], f32)
                        for fc in range(NFT):
                            nc.tensor.matmul(
                                ps2[:],
                                lhsT=h_sb[:, fc * TB + s * P: fc * TB + s * P + P],
                                rhs=w2_sb[fc][:, dh * DH:(dh + 1) * DH],
                                start=(fc == 0),
                                stop=(fc == NFT - 1 and not has_b2),
                            )
                        if has_b2:
                            nc.tensor.matmul(
                                ps2[:],
                                lhsT=ones_sb[:, :],
                                rhs=b2_sb[:, dh * DH:(dh + 1) * DH],
                                start=False,
                                stop=True,
                            )
                        nc.scalar.mul(
                            o_sb[:, s, dh * DH:(dh + 1) * DH], ps2[:],
                            g_sb[:, gcol:gcol + 1],
                        )
                    nc.sync.dma_start(
                        out.ap()[tb * TB + s * P: tb * TB + (s + 1) * P, :],
                        o_sb[:, s, :],
                    )
    nc.compile()
    return nc


def build_nc_fp8(nb=16, tb=512, has_b1=False):
    """fp8e4 DoubleRow variant: host pre-scales w1,w2 by 16 and gt by 1/16.
    Layouts: xgT8 [D,K] fp8 (k-blocks of 128 natural), w1dr [4,128,2,F] fp8,
    w2dr [16,128,2,D] fp8, b1t [128,32] f32 (only if has_b1), gt [128,K/128]
    f32 (pre-divided by 16). out [K,D] bf16. When b1 is all-zero (the spec
    case) the relu evictions use a float-const bias, removing the b1 DMA
    from the eviction dependency chain."""
    import concourse.bacc as bacc
    import concourse.mybir as mybir
    import concourse.tile as tile

    fp8 = mybir.dt.float8e4
    bf16 = mybir.dt.bfloat16
    f32 = mybir.dt.float32
    Relu = mybir.ActivationFunctionType.Relu
    DR = mybir.MatmulPerfMode.DoubleRow

    k = nb * tb
    nts = tb // P           # token subtiles per block
    NC1 = D // 256          # 4 contraction chunks (mm1)
    NC2 = F // 256          # 16 contraction chunks (mm2)

    nc = bacc.Bacc("TRN2", target_bir_lowering=False, debug=False)
    xgT = nc.dram_tensor("xgT", [D, k], fp8, kind="ExternalInput")
    w1 = nc.dram_tensor("w1", [NC1, P, 2, F], fp8, kind="ExternalInput")
    w2 = nc.dram_tensor("w2", [NC2, P, 2, D], fp8, kind="ExternalInput")
    if has_b1:
        b1t = nc.dram_tensor("b1t", [P, NFT], f32, kind="ExternalInput")
    gt = nc.dram_tensor("gt", [P, k // P], f32, kind="ExternalInput")
    out = nc.dram_tensor("out", [k, D], bf16, kind="ExternalOutput")

    with tile.TileContext(nc) as tc:
        with (
            tc.tile_pool(name="wpool", bufs=1) as wpool,
            tc.tile_pool(name="xpool", bufs=3) as xpool,
            tc.tile_pool(name="hpool", bufs=2) as hpool,
            tc.tile_pool(name="opool", bufs=3) as opool,
            tc.tile_pool(name="pp1", bufs=6, space="PSUM") as pp1,
            tc.tile_pool(name="pp2", bufs=2, space="PSUM") as pp2,
        ):
            # Warm the ACT engine's function tables (Relu for mm1 evictions,
            # Copy for the gate-scale evictions) during the DMA head — the
            # first use of an activation function pays a ~2us table load that
            # otherwise backs up PSUM and stalls the PE early on.
            warm = wpool.tile([1, 1], f32, tag="warm")
            warm_b = wpool.tile([1, 1], f32, tag="warm_b")
            nc.vector.memset(warm[:], 0.0)
            nc.vector.memset(warm_b[:], 0.0)
            nc.scalar.activation(warm[:], warm[:], Relu, bias=warm_b[:, :1])
            nc.scalar.mul(warm[:], warm[:], 1.0)

            # Explicit memset zero-bias for the relu evictions: a 0.0 float
            # bias would go through bass's const pool, whose backing DMA
            # lands at the END of the sync queue — evictions would then wait
            # behind the whole w2 load (~10us stall on the first block).
            zbias = wpool.tile([P, 1], f32, tag="zbias")
            nc.vector.memset(zbias[:], 0.0)

            # xgT rows: chunk c covers D-rows [c*256, (c+1)*256); slot j holds
            # rows c*256 + j*128 + p  ->  "(c j p) t"
            xgT_r = xgT.ap().rearrange("(c j p) t -> p c j t", j=2, p=P)

            def load_xg(tbi):
                t = xpool.tile([P, NC1, 2, tb], fp8, tag="xg")
                nc.sync.dma_start(t[:], xgT_r[:, :, :, tbi * tb:(tbi + 1) * tb])
                return t

            # DMA issue order matters for the pipeline head (descriptors drain
            # in order per queue at ~320GB/s): sync queue carries xg0 then w1
            # (first matmul needs exactly these) then xg1, then w2 (only
            # needed ~27us in). The tiny b1/g (mm1 evictions need them early)
            # go on the gpsimd queue so they don't add head latency.
            if has_b1:
                b1_sb = wpool.tile([P, NFT], f32, tag="b1")
                nc.gpsimd.dma_start(b1_sb[:], b1t.ap())
            g_sb = wpool.tile([P, k // P], f32, tag="g")
            nc.gpsimd.dma_start(g_sb[:], gt.ap())
            xg_pre = [load_xg(0)]
            w1_sb = []
            for c in range(NC1):
                t = wpool.tile([P, 2, F], fp8, tag=f"w1_{c}")
                nc.sync.dma_start(t[:], w1.ap()[c])
                w1_sb.append(t)
            if nb > 1:
                xg_pre.append(load_xg(1))
            w2_sb = []
            for c in range(NC2):
                t = wpool.tile([P, 2, D], fp8, tag=f"w2_{c}")
                nc.sync.dma_start(t[:], w2.ap()[c])
                w2_sb.append(t)


            for tbi in range(nb):
                xg_sb = xg_pre[tbi] if tbi < len(xg_pre) else load_xg(tbi)

                h_sb = hpool.tile([P, NFT, tb], fp8, tag="h")
                for ft in range(NFT):
                    ps = pp1.tile([P, tb], f32)
                    for c in range(NC1):
                        nc.tensor.matmul(
                            ps[:],
                            lhsT=w1_sb[c][:, :, ft * P:(ft + 1) * P],
                            rhs=xg_sb[:, c, :, :],
                            start=(c == 0),
                            stop=(c == NC1 - 1),
                            perf_mode=DR,
                        )
                    # psum holds 16*(x@w1); relu((psum/16) + b1)
                    nc.scalar.activation(
                        h_sb[:, ft, :], ps[:], Relu,
                        bias=(b1_sb[:, ft:ft + 1] if has_b1 else zbias[:, :1]),
                        scale=1.0 / 16.0,
                    )

                o_sb = opool.tile([P, nts, D], bf16, tag="o")
                for s in range(nts):
                    gcol = tbi * nts + s
                    for dh in range(NDH):
                        ps2 = pp2.tile([P, DH], f32)
                        for fc in range(NC2):
                            # lhsT: hT rows fc*256 + j*128 + p = F-tiles (2fc, 2fc+1)
                            nc.tensor.matmul(
                                ps2[:],
                                lhsT=h_sb[:, 2 * fc:2 * fc + 2, s * P:s * P + P],
                                rhs=w2_sb[fc][:, :, dh * DH:(dh + 1) * DH],
                                start=(fc == 0),
                                stop=(fc == NC2 - 1),
                                perf_mode=DR,
                            )
                        # psum holds 16*(h@w2); gt is pre-divided by 16
                        nc.scalar.mul(
                            o_sb[:, s, dh * DH:(dh + 1) * DH], ps2[:],
                            g_sb[:, gcol:gcol + 1],
                        )
                    nc.sync.dma_start(
                        out.ap()[tbi * tb + s * P: tbi * tb + (s + 1) * P, :],
                        o_sb[:, s, :],
                    )
    nc.compile()
    return nc


def _route_host(x, noise, w_route, b_route, w_noise, b_noise, top_k):
    """Replicates the oracle's router bit-exactly on CPU jax (op-for-op)."""
    import jax
    import jax.numpy as jnp

    cpu = jax.devices("cpu")[0]
    with jax.default_device(cpu):
        flat = jnp.asarray(np.asarray(x, np.float32)).reshape(-1, D)
        logits = (flat @ jnp.asarray(np.asarray(w_route, np.float32))
                  + jnp.asarray(np.asarray(b_route, np.float32))).T
        noise_logits = (flat @ jnp.asarray(np.asarray(w_noise, np.float32))
                        + jnp.asarray(np.asarray(b_noise, np.float32))).T
        noisy = logits + jnp.asarray(np.asarray(noise, np.float32)) * jax.nn.softplus(noise_logits)
        top_v, idx = jax.lax.top_k(noisy, top_k)
        gate = jax.nn.softmax(top_v, axis=-1)
        return np.asarray(idx), np.asarray(gate, np.float32)


def _gather_transpose(flat, idx):
    """[E,K] gather from flat [N,D] -> xgT [E, D, K] f32, via CPU jax."""
    import jax
    import jax.numpy as jnp

    cpu = jax.devices("cpu")[0]
    with jax.default_device(cpu):
        xg = jnp.take(jnp.asarray(flat), jnp.asarray(idx), axis=0)  # [E, K, D]
        xgT = jnp.transpose(xg, (0, 2, 1))
        return np.asarray(xgT)


def prepare(x, noise, w_route, b_route, w_noise, b_noise, w1, b1, w2, b2, top_k):
    """Host-side routing + sharding. Returns (build_key, in_maps, idx, flat)."""
    x = np.asarray(x, np.float32)
    w1 = np.asarray(w1, np.float32)
    b1 = np.asarray(b1, np.float32)
    w2 = np.asarray(w2, np.float32)
    b2 = np.asarray(b2, np.float32)
    assert int(top_k) == K

    idx, gate = _route_host(x, noise, w_route, b_route, w_noise, b_noise, int(top_k))
    flat = x.reshape(-1, D)
    xgT = _gather_transpose(flat, idx)

    has_b2 = bool(np.any(b2))
    has_b1 = bool(np.any(b1))
    in_maps = []
    if not has_b2:
        # fp8e4 DoubleRow path (w1,w2 pre-scaled by 16; gate divided by 16)
        key = "fp8_b1" if has_b1 else "fp8"
        f8 = ml_dtypes.float8_e4m3

        def to_f8(a):
            # clip to TRN fp8e4's +-240 range so outliers saturate, not inf
            return np.clip(a, -240.0, 240.0).astype(f8)

        for e in range(E):
            m = {
                "xgT": to_f8(xgT[e]),
                "w1": np.ascontiguousarray(
                    to_f8(w1[e] * 16).reshape(4, 2, P, F).transpose(0, 2, 1, 3)),
                "w2": np.ascontiguousarray(
                    to_f8(w2[e] * 16).reshape(16, 2, P, D).transpose(0, 2, 1, 3)),
                "gt": np.ascontiguousarray(
                    (gate[e] / 16.0).reshape(K // P, P).T.astype(np.float32)),
            }
            if has_b1:
                m["b1t"] = np.ascontiguousarray(
                    b1[e].reshape(NFT, P).T.astype(np.float32))
            in_maps.append(m)
    else:
        key = "bf16_b2"
        bf = ml_dtypes.bfloat16
        for e in range(E):
            m = {
                "xgT": xgT[e].astype(bf),
                "w1": w1[e].astype(bf),
                "w2": w2[e].astype(bf),
                "b1t": np.ascontiguousarray(b1[e].reshape(NFT, P).T.astype(np.float32)),
                "gt": np.ascontiguousarray(gate[e].reshape(K // P, P).T.astype(np.float32)),
            }
            m["b2r"] = b2[e].reshape(1, D).astype(bf)
            in_maps.append(m)

    return key, in_maps, idx, flat


def build_for(key):
    if key not in _STATE:
        if key == "fp8":
            _STATE[key] = build_nc_fp8(nb=16, tb=512, has_b1=False)
        elif key == "fp8_b1":
            _STATE[key] = build_nc_fp8(nb=16, tb=512, has_b1=True)
        else:
            _STATE[key] = build_nc(True)
    return _STATE[key]


def _run_device_subprocess(key, in_maps):
    """Disaster-recovery path: a device execution failure poisons the PJRT
    client for the rest of the process, but a fresh process's first
    execution recovers. Ship the per-core inputs to a new interpreter."""
    import os
    import subprocess
    import sys
    import tempfile

    tmp = tempfile.mkdtemp()
    inp, outp = os.path.join(tmp, "in.npz"), os.path.join(tmp, "out.npz")
    save = {}
    for e, m in enumerate(in_maps):
        for name, arr in m.items():
            dt = str(arr.dtype)
            save[f"{e}|{name}|{dt}"] = (
                arr if arr.dtype == np.float32 else arr.view(np.uint8))
    np.savez(inp, **save)
    kdir = os.path.dirname(os.path.abspath(__file__))
    runner = f"""
import numpy as np, ml_dtypes, sys
sys.path.insert(0, {kdir!r})
import kernel as kmod
from concourse.bass_utils import run_bass_kernel_spmd
z = np.load({inp!r})
in_maps = [dict() for _ in range(kmod.E)]
for kk in z.files:
    e, name, dt = kk.split('|')
    a = z[kk]
    if dt != 'float32':
        a = a.view(getattr(ml_dtypes, dt))
    in_maps[int(e)][name] = a
nc = kmod.build_for({key!r})
res = run_bass_kernel_spmd(nc, in_maps, core_ids=list(range(kmod.E)))
np.savez({outp!r}, **{{str(e): np.asarray(res.results[e]['out']).view(np.uint8)
                      for e in range(kmod.E)}})
"""
    subprocess.run([sys.executable, "-c", runner], check=True)
    z = np.load(outp)
    return [{"out": z[str(e)].view(ml_dtypes.bfloat16)} for e in range(E)]


def run_device(key, in_maps):
    from concourse.bass_utils import run_bass_kernel_spmd

    nc = build_for(key)
    for _ in range(2):
        try:
            return run_bass_kernel_spmd(nc, in_maps, core_ids=list(range(E))).results
        except Exception:
            pass
    return _run_device_subprocess(key, in_maps)


def kernel(x, noise, w_route, b_route, w_noise, b_noise, w1, b1, w2, b2, top_k):
    x = np.asarray(x, np.float32)
    B, S, _ = x.shape
    key, in_maps, idx, flat = prepare(
        x, noise, w_route, b_route, w_noise, b_noise, w1, b1, w2, b2, top_k)

    results = run_device(key, in_maps)

    final = flat.copy()
    for e in range(E):
        final[idx[e]] += np.asarray(results[e]["out"], dtype=np.float32)
    return final.reshape(B, S, D), idx


# revision 42
# speedup vs baseline: 1.0062x; 1.0005x over previous
"""Expert-choice token-sparse MoE for Trainium2 (8 NeuronCores, expert-parallel).

Contract: kernel(**inputs) takes the FULL unsharded inputs and returns the FULL
output, matching reference.reference(): a tuple (final [B,S,D] f32, idx [E,K] i32).

Sharding strategy (expert-parallel, per the hint):
  - The router + top-k runs on host CPU (bit-exact replication of the oracle's
    jax ops — idx is an integer output that must match exactly, which requires
    bit-identical fp32 routing scores; the heavy compute is NOT here).
  - Expert e's gathered tokens + weights are shipped to core e ("all-to-all
    dispatch" done host-side as part of sharding).
  - Each core runs the dense MLP for its 8192 selected tokens on the Tensor
    engine in fp8e4 DoubleRow (2 fp8 MACs/cell/cycle):
    hT = relu(w1^T @ xgT + b1); out = (hT^T @ w2) * gate  (~137 GFLOP/core).
    The fp8 quantization error is attenuated by the gate (~1e-4) relative to
    the fp32 residual, so final relative error stays ~2e-5. A bf16 variant
    handles the (spec-wise impossible) b2 != 0 case.
  - Host combines: scatter-add each expert's compact [K, D] output + residual.
Measured: ~911 us HW exec per core (92% MFU; fp8 roofline for the 137 GFLOP
is ~874 us + ~14 us DMA head + ~12 us drain tail).
"""

import numpy as np
import ml_dtypes

E = 8
D = 1024
F = 4096
N = 32768
K = 8192
P = 128
TB = 256              # tokens per device block
NDC = D // P          # 8 d-chunks
NFT = F // P          # 32 f-tiles
NB = K // TB          # 32 token blocks
NTS = TB // P         # 2 token subtiles per block
DH = 512              # out D half (PSUM bank)
NDH = D // DH         # 2

_STATE = {}


def build_nc(has_b2=False, nb=NB):
    import concourse.bacc as bacc
    import concourse.mybir as mybir
    import concourse.tile as tile

    bf16 = mybir.dt.bfloat16
    f32 = mybir.dt.float32
    Relu = mybir.ActivationFunctionType.Relu

    k = nb * TB
    nc = bacc.Bacc("TRN2", target_bir_lowering=False, debug=False)
    xgT = nc.dram_tensor("xgT", [D, k], bf16, kind="ExternalInput")
    w1 = nc.dram_tensor("w1", [D, F], bf16, kind="ExternalInput")
    w2 = nc.dram_tensor("w2", [F, D], bf16, kind="ExternalInput")
    b1t = nc.dram_tensor("b1t", [P, NFT], f32, kind="ExternalInput")
    gt = nc.dram_tensor("gt", [P, k // P], f32, kind="ExternalInput")
    if has_b2:
        b2r = nc.dram_tensor("b2r", [1, D], bf16, kind="ExternalInput")
    out = nc.dram_tensor("out", [k, D], bf16, kind="ExternalOutput")

    with tile.TileContext(nc) as tc:
        with (
            tc.tile_pool(name="wpool", bufs=1) as wpool,
            tc.tile_pool(name="xpool", bufs=3) as xpool,
            tc.tile_pool(name="hpool", bufs=2) as hpool,
            tc.tile_pool(name="opool", bufs=3) as opool,
            tc.tile_pool(name="pp1", bufs=4, space="PSUM") as pp1,
            tc.tile_pool(name="pp2", bufs=4, space="PSUM") as pp2,
        ):
            # resident weights
            w1_sb = []
            for c in range(NDC):
                t = wpool.tile([P, F], bf16, tag=f"w1_{c}")
                nc.sync.dma_start(t[:], w1.ap()[c * P:(c + 1) * P, :])
                w1_sb.append(t)
            w2_sb = []
            for c in range(NFT):
                t = wpool.tile([P, D], bf16, tag=f"w2_{c}")
                nc.sync.dma_start(t[:], w2.ap()[c * P:(c + 1) * P, :])
                w2_sb.append(t)
            b1_sb = wpool.tile([P, NFT], f32, tag="b1")
            nc.sync.dma_start(b1_sb[:], b1t.ap())
            g_sb = wpool.tile([P, k // P], f32, tag="g")
            nc.sync.dma_start(g_sb[:], gt.ap())
            if has_b2:
                b2_sb = wpool.tile([1, D], bf16, tag="b2")
                nc.sync.dma_start(b2_sb[:], b2r.ap())
                ones_sb = wpool.tile([1, P], bf16, tag="ones")
                nc.vector.memset(ones_sb[:], 1.0)

            xgT_r = xgT.ap().rearrange("(c p) t -> p c t", p=P)

            for tb in range(nb):
                xg_sb = xpool.tile([P, NDC, TB], bf16, tag="xg")
                nc.sync.dma_start(
                    xg_sb[:], xgT_r[:, :, tb * TB:(tb + 1) * TB]
                )

                h_sb = hpool.tile([P, NFT * TB], bf16, tag="h")
                for ft in range(NFT):
                    ps = pp1.tile([P, TB], f32)
                    for c in range(NDC):
                        nc.tensor.matmul(
                            ps[:],
                            lhsT=w1_sb[c][:, ft * P:(ft + 1) * P],
                            rhs=xg_sb[:, c, :],
                            start=(c == 0),
                            stop=(c == NDC - 1),
                        )
                    nc.scalar.activation(
                        h_sb[:, ft * TB:(ft + 1) * TB], ps[:], Relu,
                        bias=b1_sb[:, ft:ft + 1],
                    )

                o_sb = opool.tile([P, NTS, D], bf16, tag="o")
                for s in range(NTS):
                    gcol = tb * NTS + s
                    for dh in range(NDH):
                        ps2 = pp2.tile([P, DH], f32)
                        for fc in range(NFT):
                            nc.tensor.matmul(
                                ps2[:],
                                lhsT=h_sb[:, fc * TB + s * P: fc * TB + s * P + P],
                                rhs=w2_sb[fc][:, dh * DH:(dh + 1) * DH],
                                start=(fc == 0),
                                stop=(fc == NFT - 1 and not has_b2),
                            )
                        if has_b2:
                            nc.tensor.matmul(
                                ps2[:],
                                lhsT=ones_sb[:, :],
                                rhs=b2_sb[:, dh * DH:(dh + 1) * DH],
                                start=False,
                                stop=True,
                            )
                        nc.scalar.mul(
                            o_sb[:, s, dh * DH:(dh + 1) * DH], ps2[:],
                            g_sb[:, gcol:gcol + 1],
                        )
                    nc.sync.dma_start(
                        out.ap()[tb * TB + s * P: tb * TB + (s + 1) * P, :],
                        o_sb[:, s, :],
                    )
    nc.compile()
    return nc


def build_nc_fp8(nb=16, tb=512, has_b1=False):
    """fp8e4 DoubleRow variant: host pre-scales w1,w2 by 16 and gt by 1/16.
    Layouts: xgT8 [D,K] fp8 (k-blocks of 128 natural), w1dr [4,128,2,F] fp8,
    w2dr [16,128,2,D] fp8, b1t [128,32] f32 (only if has_b1), gt [128,K/128]
    f32 (pre-divided by 16). out [K,D] bf16. When b1 is all-zero (the spec
    case) the relu evictions use a float-const bias, removing the b1 DMA
    from the eviction dependency chain."""
    import concourse.bacc as bacc
    import concourse.mybir as mybir
    import concourse.tile as tile

    fp8 = mybir.dt.float8e4
    bf16 = mybir.dt.bfloat16
    f32 = mybir.dt.float32
    Relu = mybir.ActivationFunctionType.Relu
    DR = mybir.MatmulPerfMode.DoubleRow

    k = nb * tb
    nts = tb // P           # token subtiles per block
    NC1 = D // 256          # 4 contraction chunks (mm1)
    NC2 = F // 256          # 16 contraction chunks (mm2)

    nc = bacc.Bacc("TRN2", target_bir_lowering=False, debug=False)
    xgT = nc.dram_tensor("xgT", [D, k], fp8, kind="ExternalInput")
    w1 = nc.dram_tensor("w1", [NC1, P, 2, F], fp8, kind="ExternalInput")
    w2 = nc.dram_tensor("w2", [NC2, P, 2, D], fp8, kind="ExternalInput")
    if has_b1:
        b1t = nc.dram_tensor("b1t", [P, NFT], f32, kind="ExternalInput")
    gt = nc.dram_tensor("gt", [P, k // P], f32, kind="ExternalInput")
    out = nc.dram_tensor("out", [k, D], bf16, kind="ExternalOutput")

    with tile.TileContext(nc) as tc:
        with (
            tc.tile_pool(name="wpool", bufs=1) as wpool,
            tc.tile_pool(name="xpool", bufs=3) as xpool,
            tc.tile_pool(name="hpool", bufs=2) as hpool,
            tc.tile_pool(name="opool", bufs=3) as opool,
            tc.tile_pool(name="pp1", bufs=6, space="PSUM") as pp1,
            tc.tile_pool(name="pp2", bufs=2, space="PSUM") as pp2,
        ):
            # Warm the ACT engine's function tables (Relu for mm1 evictions,
            # Copy for the gate-scale evictions) during the DMA head — the
            # first use of an activation function pays a ~2us table load that
            # otherwise backs up PSUM and stalls the PE early on.
            warm = wpool.tile([1, 1], f32, tag="warm")
            warm_b = wpool.tile([1, 1], f32, tag="warm_b")
            nc.vector.memset(warm[:], 0.0)
            nc.vector.memset(warm_b[:], 0.0)
            nc.scalar.activation(warm[:], warm[:], Relu, bias=warm_b[:, :1])
            nc.scalar.mul(warm[:], warm[:], 1.0)

            # Explicit memset zero-bias for the relu evictions: a 0.0 float
            # bias would go through bass's const pool, whose backing DMA
            # lands at the END of the sync queue — evictions would then wait
            # behind the whole w2 load (~10us stall on the first block).
            zbias = wpool.tile([P, 1], f32, tag="zbias")
            nc.vector.memset(zbias[:], 0.0)

            # xgT rows: chunk c covers D-rows [c*256, (c+1)*256); slot j holds
            # rows c*256 + j*128 + p  ->  "(c j p) t"
            xgT_r = xgT.ap().rearrange("(c j p) t -> p c j t", j=2, p=P)

            def load_xg(tbi):
                t = xpool.tile([P, NC1, 2, tb], fp8, tag="xg")
                nc.sync.dma_start(t[:], xgT_r[:, :, :, tbi * tb:(tbi + 1) * tb])
                return t

            # DMA issue order matters for the pipeline head (descriptors drain
            # in order per queue at ~320GB/s): sync queue carries xg0 then w1
            # (first matmul needs exactly these) then xg1, then w2 (only
            # needed ~27us in). The tiny b1/g (mm1 evictions need them early)
            # go on the gpsimd queue so they don't add head latency.
            if has_b1:
                b1_sb = wpool.tile([P, NFT], f32, tag="b1")
                nc.gpsimd.dma_start(b1_sb[:], b1t.ap())
            g_sb = wpool.tile([P, k // P], f32, tag="g")
            nc.gpsimd.dma_start(g_sb[:], gt.ap())
            xg_pre = [load_xg(0)]
            w1_sb = []
            for c in range(NC1):
                t = wpool.tile([P, 2, F], fp8, tag=f"w1_{c}")
                nc.sync.dma_start(t[:], w1.ap()[c])
                w1_sb.append(t)
            if nb > 1:
                xg_pre.append(load_xg(1))
            w2_sb = []
            for c in range(NC2):
                t = wpool.tile([P, 2, D], fp8, tag=f"w2_{c}")
                nc.sync.dma_start(t[:], w2.ap()[c])
                w2_sb.append(t)


            for tbi in range(nb):
                xg_sb = xg_pre[tbi] if tbi < len(xg_pre) else load_xg(tbi)

                h_sb = hpool.tile([P, NFT, tb], fp8, tag="h")
                for ft in range(NFT):
                    ps = pp1.tile([P, tb], f32)
                    for c in range(NC1):
                        nc.tensor.matmul(
                            ps[:],
                            lhsT=w1_sb[c][:, :, ft * P:(ft + 1) * P],
                            rhs=xg_sb[:, c, :, :],
                            start=(c == 0),
                            stop=(c == NC1 - 1),
                            perf_mode=DR,
                        )
                    # psum holds 16*(x@w1); relu((psum/16) + b1)
                    nc.scalar.activation(
                        h_sb[:, ft, :], ps[:], Relu,
                        bias=(b1_sb[:, ft:ft + 1] if has_b1 else zbias[:, :1]),
                        scale=1.0 / 16.0,
                    )

                o_sb = opool.tile([P, nts, D], bf16, tag="o")
                for s in range(nts):
                    gcol = tbi * nts + s
                    for dh in range(NDH):
                        ps2 = pp2.tile([P, DH], f32)
                        for fc in range(NC2):
                            # lhsT: hT rows fc*256 + j*128 + p = F-tiles (2fc, 2fc+1)
                            nc.tensor.matmul(
                                ps2[:],
                                lhsT=h_sb[:, 2 * fc:2 * fc + 2, s * P:s * P + P],
                                rhs=w2_sb[fc][:, :, dh * DH:(dh + 1) * DH],
                                start=(fc == 0),
                                stop=(fc == NC2 - 1),
                                perf_mode=DR,
                            )
                        # psum holds 16*(h@w2); gt is pre-divided by 16.
                        # DVE (not ACT) so these evictions never queue ahead
                        # of the next block's relu evictions in the ACT FIFO
                        nc.vector.tensor_scalar_mul(
                            o_sb[:, s, dh * DH:(dh + 1) * DH], ps2[:],
                            g_sb[:, gcol:gcol + 1],
                        )
                    nc.sync.dma_start(
                        out.ap()[tbi * tb + s * P: tbi * tb + (s + 1) * P, :],
                        o_sb[:, s, :],
                    )
    nc.compile()
    return nc


def _route_host(x, noise, w_route, b_route, w_noise, b_noise, top_k):
    """Replicates the oracle's router bit-exactly on CPU jax (op-for-op)."""
    import jax
    import jax.numpy as jnp

    cpu = jax.devices("cpu")[0]
    with jax.default_device(cpu):
        flat = jnp.asarray(np.asarray(x, np.float32)).reshape(-1, D)
        logits = (flat @ jnp.asarray(np.asarray(w_route, np.float32))
                  + jnp.asarray(np.asarray(b_route, np.float32))).T
        noise_logits = (flat @ jnp.asarray(np.asarray(w_noise, np.float32))
                        + jnp.asarray(np.asarray(b_noise, np.float32))).T
        noisy = logits + jnp.asarray(np.asarray(noise, np.float32)) * jax.nn.softplus(noise_logits)
        top_v, idx = jax.lax.top_k(noisy, top_k)
        gate = jax.nn.softmax(top_v, axis=-1)
        return np.asarray(idx), np.asarray(gate, np.float32)


def _gather_transpose(flat, idx):
    """[E,K] gather from flat [N,D] -> xgT [E, D, K] f32, via CPU jax."""
    import jax
    import jax.numpy as jnp

    cpu = jax.devices("cpu")[0]
    with jax.default_device(cpu):
        xg = jnp.take(jnp.asarray(flat), jnp.asarray(idx), axis=0)  # [E, K, D]
        xgT = jnp.transpose(xg, (0, 2, 1))
        return np.asarray(xgT)


def prepare(x, noise, w_route, b_route, w_noise, b_noise, w1, b1, w2, b2, top_k):
    """Host-side routing + sharding. Returns (build_key, in_maps, idx, flat)."""
    x = np.asarray(x, np.float32)
    w1 = np.asarray(w1, np.float32)
    b1 = np.asarray(b1, np.float32)
    w2 = np.asarray(w2, np.float32)
    b2 = np.asarray(b2, np.float32)
    assert int(top_k) == K

    idx, gate = _route_host(x, noise, w_route, b_route, w_noise, b_noise, int(top_k))
    flat = x.reshape(-1, D)
    xgT = _gather_transpose(flat, idx)

    has_b2 = bool(np.any(b2))
    has_b1 = bool(np.any(b1))
    in_maps = []
    if not has_b2:
        # fp8e4 DoubleRow path (w1,w2 pre-scaled by 16; gate divided by 16)
        key = "fp8_b1" if has_b1 else "fp8"
        f8 = ml_dtypes.float8_e4m3

        def to_f8(a):
            # clip to TRN fp8e4's +-240 range so outliers saturate, not inf
            return np.clip(a, -240.0, 240.0).astype(f8)

        for e in range(E):
            m = {
                "xgT": to_f8(xgT[e]),
                "w1": np.ascontiguousarray(
                    to_f8(w1[e] * 16).reshape(4, 2, P, F).transpose(0, 2, 1, 3)),
                "w2": np.ascontiguousarray(
                    to_f8(w2[e] * 16).reshape(16, 2, P, D).transpose(0, 2, 1, 3)),
                "gt": np.ascontiguousarray(
                    (gate[e] / 16.0).reshape(K // P, P).T.astype(np.float32)),
            }
            if has_b1:
                m["b1t"] = np.ascontiguousarray(
                    b1[e].reshape(NFT, P).T.astype(np.float32))
            in_maps.append(m)
    else:
        key = "bf16_b2"
        bf = ml_dtypes.bfloat16
        for e in range(E):
            m = {
                "xgT": xgT[e].astype(bf),
                "w1": w1[e].astype(bf),
                "w2": w2[e].astype(bf),
                "b1t": np.ascontiguousarray(b1[e].reshape(NFT, P).T.astype(np.float32)),
                "gt": np.ascontiguousarray(gate[e].reshape(K // P, P).T.astype(np.float32)),
            }
            m["b2r"] = b2[e].reshape(1, D).astype(bf)
            in_maps.append(m)

    return key, in_maps, idx, flat


def build_for(key):
    if key not in _STATE:
        if key == "fp8":
            _STATE[key] = build_nc_fp8(nb=16, tb=512, has_b1=False)
        elif key == "fp8_b1":
            _STATE[key] = build_nc_fp8(nb=16, tb=512, has_b1=True)
        else:
            _STATE[key] = build_nc(True)
    return _STATE[key]


def _run_device_subprocess(key, in_maps):
    """Disaster-recovery path: a device execution failure poisons the PJRT
    client for the rest of the process, but a fresh process's first
    execution recovers. Ship the per-core inputs to a new interpreter."""
    import os
    import subprocess
    import sys
    import tempfile

    tmp = tempfile.mkdtemp()
    inp, outp = os.path.join(tmp, "in.npz"), os.path.join(tmp, "out.npz")
    save = {}
    for e, m in enumerate(in_maps):
        for name, arr in m.items():
            dt = str(arr.dtype)
            save[f"{e}|{name}|{dt}"] = (
                arr if arr.dtype == np.float32 else arr.view(np.uint8))
    np.savez(inp, **save)
    kdir = os.path.dirname(os.path.abspath(__file__))
    runner = f"""
import numpy as np, ml_dtypes, sys
sys.path.insert(0, {kdir!r})
import kernel as kmod
from concourse.bass_utils import run_bass_kernel_spmd
z = np.load({inp!r})
in_maps = [dict() for _ in range(kmod.E)]
for kk in z.files:
    e, name, dt = kk.split('|')
    a = z[kk]
    if dt != 'float32':
        a = a.view(getattr(ml_dtypes, dt))
    in_maps[int(e)][name] = a
nc = kmod.build_for({key!r})
res = run_bass_kernel_spmd(nc, in_maps, core_ids=list(range(kmod.E)))
np.savez({outp!r}, **{{str(e): np.asarray(res.results[e]['out']).view(np.uint8)
                      for e in range(kmod.E)}})
"""
    subprocess.run([sys.executable, "-c", runner], check=True)
    z = np.load(outp)
    return [{"out": z[str(e)].view(ml_dtypes.bfloat16)} for e in range(E)]


def run_device(key, in_maps):
    from concourse.bass_utils import run_bass_kernel_spmd

    nc = build_for(key)
    for _ in range(2):
        try:
            return run_bass_kernel_spmd(nc, in_maps, core_ids=list(range(E))).results
        except Exception:
            pass
    return _run_device_subprocess(key, in_maps)


def kernel(x, noise, w_route, b_route, w_noise, b_noise, w1, b1, w2, b2, top_k):
    x = np.asarray(x, np.float32)
    B, S, _ = x.shape
    key, in_maps, idx, flat = prepare(
        x, noise, w_route, b_route, w_noise, b_noise, w1, b1, w2, b2, top_k)

    results = run_device(key, in_maps)

    final = flat.copy()
    for e in range(E):
        final[idx[e]] += np.asarray(results[e]["out"], dtype=np.float32)
    return final.reshape(B, S, D), idx
